# revision 1
# baseline (speedup 1.0000x reference)
"""InteractionNetwork (GNN message passing) Bass kernel for 8 Trainium2 cores.

Strategy (edge-sharded, per sharding hint):
  - Shard the 32768 edges across 8 cores (4096 each). Each core streams its
    rr/rs one-hot slices from HBM exactly once (the memory roofline),
    fp32->fp16 cast in the DMA.
  - Receiver/sender indices are recovered on-device with a one-hot dot iota:
    VectorE tensor_tensor multiply, then the free-dim sum split across
    ScalarE (activation accum_out, rr) and VectorE (tensor_reduce, rs) to
    balance engine load; node features are gathered with indirect DMA; the
    4-layer relation MLP runs feature-major on the PE; edge effects are
    aggregated to nodes with a natural-layout matmul
    e_agg.T += e_chunk.T @ rr_chunk  into a pinned PSUM accumulator.
    Per-128-edge-chunk DMAs with 8-deep buffering pipeline index/marshal
    work of group g+1 under group g's MLP/aggregation.
  - Partial e_agg is AllReduce-summed across the 8 cores; every core then
    runs the small object MLP on all 2048 nodes; host takes core 0's output.
"""

import os
import sys

import numpy as np

os.environ.setdefault("MYCRO_LOCAL_CACHE", "1")
for _p in ("/opt/trn_rl_repo",):
    if os.path.isdir(_p) and _p not in sys.path:
        sys.path.insert(0, _p)

import concourse.bacc as bacc
import concourse.bass as bass
import concourse.mybir as mybir
import concourse.tile as tile
from concourse.bass_utils import run_bass_kernel_spmd
from concourse.masks import make_identity

P = 128
F32 = mybir.dt.float32
F16 = mybir.dt.float16
I32 = mybir.dt.int32
I16 = mybir.dt.int16
AF = mybir.ActivationFunctionType
ALU = mybir.AluOpType

N_OBJ, N_REL = 2048, 32768
D_OBJ, D_REL, D_EFF = 64, 32, 64
H_REL, H_OBJ = 128, 128
D_OUT = 3
N_CORES = 8


def build(n_cores=N_CORES, e_per_core=N_REL // N_CORES, n_obj=N_OBJ,
          use_collective=True, use_indirect=True, use_ttr=True,
          sbufs=8, gbufs=4, rs_accum_dve=True):
    EG = 512                  # edges per MLP group
    T = EG // P               # 128-edge chunks per group
    n_groups = e_per_core // EG
    NQ = 512                  # node chunk (psum bank) for wide matmuls
    n_nq = n_obj // NQ

    nc = bacc.Bacc(
        "TRN2",
        target_bir_lowering=False,
        debug=False,
        enable_asserts=False,
        num_devices=n_cores,
    )

    rr = nc.dram_tensor("rr_c", [e_per_core, n_obj], F32, kind="ExternalInput")
    rs = nc.dram_tensor("rs_c", [e_per_core, n_obj], F32, kind="ExternalInput")
    ra = nc.dram_tensor("ra_c", [e_per_core, D_REL], F32, kind="ExternalInput")
    obj = nc.dram_tensor("obj", [n_obj, D_OBJ], F32, kind="ExternalInput")
    rm_w1 = nc.dram_tensor("rm_w1", [2 * D_OBJ + D_REL, H_REL], F32, kind="ExternalInput")
    rm_b1 = nc.dram_tensor("rm_b1", [H_REL], F32, kind="ExternalInput")
    rm_w2 = nc.dram_tensor("rm_w2", [H_REL, H_REL], F32, kind="ExternalInput")
    rm_b2 = nc.dram_tensor("rm_b2", [H_REL], F32, kind="ExternalInput")
    rm_w3 = nc.dram_tensor("rm_w3", [H_REL, H_REL], F32, kind="ExternalInput")
    rm_b3 = nc.dram_tensor("rm_b3", [H_REL], F32, kind="ExternalInput")
    rm_w4 = nc.dram_tensor("rm_w4", [H_REL, D_EFF], F32, kind="ExternalInput")
    rm_b4 = nc.dram_tensor("rm_b4", [D_EFF], F32, kind="ExternalInput")
    om_w1 = nc.dram_tensor("om_w1", [D_OBJ + D_EFF, H_OBJ], F32, kind="ExternalInput")
    om_b1 = nc.dram_tensor("om_b1", [H_OBJ], F32, kind="ExternalInput")
    om_w2 = nc.dram_tensor("om_w2", [H_OBJ, D_OUT], F32, kind="ExternalInput")
    om_b2 = nc.dram_tensor("om_b2", [D_OUT], F32, kind="ExternalInput")
    pT_d = nc.dram_tensor("pT", [D_OUT, n_obj], F32, kind="ExternalOutput")

    with tile.TileContext(nc) as tc:
        with (
            tc.tile_pool(name="const", bufs=1) as const,
            tc.tile_pool(name="stream", bufs=sbufs) as sp,
            tc.tile_pool(name="gat", bufs=gbufs) as gp,
            tc.tile_pool(name="ec", bufs=8) as ecp,
            tc.tile_pool(name="aggp", bufs=1, space="PSUM") as aggp,
            tc.tile_pool(name="psp", bufs=4, space="PSUM") as psp,
            tc.tile_pool(name="dram", bufs=1, space="DRAM") as dp,
        ):
            # ---- constants -------------------------------------------------
            ident32 = const.tile([P, P], F32)
            make_identity(nc, ident32[:])
            ident16 = const.tile([P, P], F16)
            make_identity(nc, ident16[:])

            iota_i = const.tile([P, n_obj], I16)
            nc.gpsimd.iota(iota_i[:], pattern=[[1, n_obj]], base=0, channel_multiplier=0)
            iota16 = const.tile([P, n_obj], F16)
            nc.vector.tensor_copy(iota16[:], iota_i[:])

            w1ab = const.tile([P, H_REL], F32)
            nc.sync.dma_start(w1ab[:], rm_w1[0:P, :])
            w1c = const.tile([D_REL, H_REL], F32)
            nc.sync.dma_start(w1c[:], rm_w1[P : P + D_REL, :])
            w2 = const.tile([H_REL, H_REL], F32)
            nc.sync.dma_start(w2[:], rm_w2[:, :])
            w3 = const.tile([H_REL, H_REL], F32)
            nc.sync.dma_start(w3[:], rm_w3[:, :])
            w4 = const.tile([H_REL, D_EFF], F32)
            nc.sync.dma_start(w4[:], rm_w4[:, :])
            b1t = const.tile([H_REL, 1], F32)
            nc.sync.dma_start(b1t[:], rm_b1[:, None])
            b2t = const.tile([H_REL, 1], F32)
            nc.sync.dma_start(b2t[:], rm_b2[:, None])
            b3t = const.tile([H_REL, 1], F32)
            nc.sync.dma_start(b3t[:], rm_b3[:, None])
            b4t = const.tile([D_EFF, 1], F32)
            nc.sync.dma_start(b4t[:], rm_b4[:, None])
            ow1a = const.tile([D_OBJ, H_OBJ], F32)
            nc.sync.dma_start(ow1a[:], om_w1[0:D_OBJ, :])
            ow1b = const.tile([D_EFF, H_OBJ], F32)
            nc.sync.dma_start(ow1b[:], om_w1[D_OBJ : D_OBJ + D_EFF, :])
            ow2 = const.tile([H_OBJ, D_OUT], F32)
            nc.sync.dma_start(ow2[:], om_w2[:, :])
            ob1t = const.tile([H_OBJ, 1], F32)
            nc.sync.dma_start(ob1t[:], om_b1[:, None])
            ob2t = const.tile([D_OUT, 1], F32)
            nc.sync.dma_start(ob2t[:], om_b2[:, None])

            # obj.T in SBUF (for the node-model MLP)
            objT = const.tile([D_OBJ, n_obj], F32)
            for k in range(n_obj // P):
                ot = gp.tile([P, D_OBJ], F32, tag="objload")
                nc.sync.dma_start(ot[:], obj[k * P : (k + 1) * P, :])
                tp = psp.tile([D_OBJ, P], F32, tag="ps")
                nc.tensor.transpose(tp[:], ot[:], ident32[:])
                nc.scalar.copy(objT[:, k * P : (k + 1) * P], tp[:])

            # pinned accumulator: e_agg.T [64, n_obj] (4 PSUM banks)
            agg_ps = aggp.tile([D_EFF, n_obj], F32)

            # ---- edge phase ------------------------------------------------
            for g in range(n_groups):
                e0 = g * EG
                rrt = []
                for t in range(T):
                    rt = sp.tile([P, n_obj], F16, tag="rrt")
                    nc.gpsimd.dma_start(rt[:], rr[e0 + t * P : e0 + (t + 1) * P, :])
                    rrt.append(rt)
                rag = sp.tile([P, T, D_REL], F32, tag="rag")
                nc.sync.dma_start(
                    rag[:], ra[e0 : e0 + EG, :].rearrange("(t p) d -> p t d", p=P)
                )

                idxf = sp.tile([P, 2 * T], F32, tag="idxf")
                idxi = sp.tile([P, 2 * T], I32, tag="idxi")
                if use_ttr:
                    for t in range(T):
                        st = sp.tile([P, n_obj], F16, tag="rst")
                        nc.gpsimd.dma_start(
                            st[:], rs[e0 + t * P : e0 + (t + 1) * P, :]
                        )
                        scr = sp.tile([P, n_obj], F16, tag="scr")
                        nc.vector.tensor_tensor(
                            out=scr[:], in0=rrt[t][:], in1=iota16[:],
                            op=ALU.mult,
                        )
                        nc.scalar.activation(
                            scr[:], scr[:], AF.Copy,
                            accum_out=idxf[:, t : t + 1],
                        )
                        nc.vector.tensor_tensor(
                            out=st[:], in0=st[:], in1=iota16[:],
                            op=ALU.mult,
                        )
                        if rs_accum_dve:
                            nc.vector.tensor_reduce(
                                out=idxf[:, T + t : T + t + 1], in_=st[:],
                                axis=mybir.AxisListType.X, op=ALU.add,
                            )
                        else:
                            nc.scalar.activation(
                                st[:], st[:], AF.Copy,
                                accum_out=idxf[:, T + t : T + t + 1],
                            )
                else:
                    nc.gpsimd.memset(idxf[:], 0.0)
                nc.vector.tensor_copy(idxi[:], idxf[:])

                b1T = sp.tile([P, EG], F32, tag="b1T")
                raT = sp.tile([D_REL, EG], F32, tag="raT")
                for t in range(T):
                    orr_t = gp.tile([P, D_OBJ], F32, tag="gat")
                    if use_indirect:
                        nc.gpsimd.indirect_dma_start(
                            out=orr_t[:], out_offset=None, in_=obj[:, :],
                            in_offset=bass.IndirectOffsetOnAxis(
                                ap=idxi[:, t : t + 1], axis=0
                            ),
                        )
                    else:
                        nc.sync.dma_start(orr_t[:], obj[0:P, :])
                    tp = psp.tile([D_OBJ, P], F32, tag="ps")
                    nc.tensor.transpose(tp[:], orr_t[:], ident32[:])
                    nc.scalar.copy(b1T[0:D_OBJ, t * P : (t + 1) * P], tp[:])

                    ors_t = gp.tile([P, D_OBJ], F32, tag="gat")
                    if use_indirect:
                        nc.gpsimd.indirect_dma_start(
                            out=ors_t[:], out_offset=None, in_=obj[:, :],
                            in_offset=bass.IndirectOffsetOnAxis(
                                ap=idxi[:, T + t : T + t + 1], axis=0
                            ),
                        )
                    else:
                        nc.sync.dma_start(ors_t[:], obj[0:P, :])
                    tp2 = psp.tile([D_OBJ, P], F32, tag="ps")
                    nc.tensor.transpose(tp2[:], ors_t[:], ident32[:])
                    nc.scalar.copy(b1T[D_OBJ : 2 * D_OBJ, t * P : (t + 1) * P], tp2[:])

                    tp3 = psp.tile([D_REL, P], F32, tag="ps")
                    nc.tensor.transpose(tp3[:], rag[:, t, :], ident32[:])
                    nc.scalar.copy(raT[:, t * P : (t + 1) * P], tp3[:])

                # relation MLP, feature-major [features, EG]
                h1p = psp.tile([H_REL, EG], F32, tag="ps")
                nc.tensor.matmul(h1p[:], w1ab[:], b1T[:], start=True, stop=False)
                nc.tensor.matmul(h1p[:], w1c[:], raT[:], start=False, stop=True)
                h1T = sp.tile([H_REL, EG], F32, tag="hT")
                nc.scalar.activation(h1T[:], h1p[:], AF.Relu, bias=b1t[:])

                h2p = psp.tile([H_REL, EG], F32, tag="ps")
                nc.tensor.matmul(h2p[:], w2[:], h1T[:], start=True, stop=True)
                h2T = sp.tile([H_REL, EG], F32, tag="hT")
                nc.scalar.activation(h2T[:], h2p[:], AF.Relu, bias=b2t[:])

                h3p = psp.tile([H_REL, EG], F32, tag="ps")
                nc.tensor.matmul(h3p[:], w3[:], h2T[:], start=True, stop=True)
                h3T = sp.tile([H_REL, EG], F32, tag="hT")
                nc.scalar.activation(h3T[:], h3p[:], AF.Relu, bias=b3t[:])

                h4p = psp.tile([D_EFF, EG], F32, tag="ps")
                nc.tensor.matmul(h4p[:], w4[:], h3T[:], start=True, stop=True)
                eT = sp.tile([D_EFF, EG], F16, tag="eT")
                nc.scalar.activation(eT[:], h4p[:], AF.Relu, bias=b4t[:])

                # aggregate: e_agg.T += e_chunk.T @ rr_chunk
                for t in range(T):
                    ep = psp.tile([P, D_EFF], F16, tag="ps")
                    nc.tensor.transpose(
                        ep[:], eT[:, t * P : (t + 1) * P], ident16[:D_EFF, :D_EFF]
                    )
                    ec = ecp.tile([P, D_EFF], F16, tag="ec")
                    nc.scalar.copy(ec[:], ep[:])
                    first = g == 0 and t == 0
                    last = g == n_groups - 1 and t == T - 1
                    for q in range(n_obj // NQ):
                        nc.tensor.matmul(
                            agg_ps[:, q * NQ : (q + 1) * NQ],
                            ec[:],
                            rrt[t][:, q * NQ : (q + 1) * NQ],
                            start=first,
                            stop=last,
                        )

            # ---- all-reduce e_agg across cores -----------------------------
            eagg_sb = const.tile([D_EFF, n_obj], F32)
            nc.scalar.copy(eagg_sb[:], agg_ps[:])
            cc_in = dp.tile([D_EFF, n_obj], F32)
            cc_out = dp.tile([D_EFF, n_obj], F32)
            nc.sync.dma_start(cc_in[:], eagg_sb[:])
            if use_collective:
                nc.gpsimd.collective_compute(
                    "AllReduce",
                    ALU.add,
                    replica_groups=[list(range(n_cores))],
                    ins=[cc_in.opt()],
                    outs=[cc_out.opt()],
                )
            else:
                nc.sync.dma_start(cc_out[:], cc_in[:])
            eaggT = const.tile([D_EFF, n_obj], F32)
            nc.sync.dma_start(eaggT[:], cc_out[:])

            # ---- node phase (object MLP) -----------------------------------
            pTt = const.tile([D_OUT, n_obj], F32)
            for q in range(n_nq):
                sl = slice(q * NQ, (q + 1) * NQ)
                cp = psp.tile([H_OBJ, NQ], F32, tag="ps")
                nc.tensor.matmul(cp[:], ow1a[:], objT[:, sl], start=True, stop=False)
                nc.tensor.matmul(cp[:], ow1b[:], eaggT[:, sl], start=False, stop=True)
                hT = sp.tile([H_OBJ, NQ], F32, tag="hT")
                nc.scalar.activation(hT[:], cp[:], AF.Relu, bias=ob1t[:])
                pp = psp.tile([D_OUT, NQ], F32, tag="ps")
                nc.tensor.matmul(pp[:], ow2[:], hT[:], start=True, stop=True)
                nc.scalar.activation(pTt[:, sl], pp[:], AF.Identity, bias=ob2t[:])
            nc.sync.dma_start(pT_d[:, :], pTt[:])

    nc.compile()
    return nc


_CACHE = {}
TRACE = False


def _get_nc():
    if "nc" not in _CACHE:
        _CACHE["nc"] = build()
    return _CACHE["nc"]


def kernel(**inputs):
    nc = _get_nc()
    f = lambda k: np.ascontiguousarray(np.asarray(inputs[k], dtype=np.float32))
    obj = f("obj")
    shared = {
        "obj": obj,
        "rm_w1": f("rm_w1"), "rm_b1": f("rm_b1"),
        "rm_w2": f("rm_w2"), "rm_b2": f("rm_b2"),
        "rm_w3": f("rm_w3"), "rm_b3": f("rm_b3"),
        "rm_w4": f("rm_w4"), "rm_b4": f("rm_b4"),
        "om_w1": f("om_w1"), "om_b1": f("om_b1"),
        "om_w2": f("om_w2"), "om_b2": f("om_b2"),
    }
    rr = f("rr")
    rs = f("rs")
    ra = f("ra")
    epc = N_REL // N_CORES
    in_maps = []
    for c in range(N_CORES):
        sl = slice(c * epc, (c + 1) * epc)
        m = dict(shared)
        m["rr_c"] = np.ascontiguousarray(rr[sl])
        m["rs_c"] = np.ascontiguousarray(rs[sl])
        m["ra_c"] = np.ascontiguousarray(ra[sl])
        in_maps.append(m)
    res = run_bass_kernel_spmd(
        nc, in_maps, core_ids=list(range(N_CORES)), trace=TRACE
    )
    _CACHE["last_results"] = res
    return np.ascontiguousarray(res.results[0]["pT"].T)



# revision 4
# speedup vs baseline: 16.3876x; 16.3876x over previous
"""InteractionNetwork (GNN message passing) Bass kernel for 8 Trainium2 cores.

Strategy (edge-sharded, per sharding hint):
  - The rr/rs one-hot matrices are a dense encoding of receiver/sender index
    vectors. The host losslessly re-encodes them as int32 indices (exact
    GEMV against an iota vector), so each call ships ~10 MB instead of
    ~540 MB of one-hot data through the PJRT tunnel.
  - Edges are sharded across 8 cores (4096 each). On device, per 128-edge
    chunk: receiver/sender node features are gathered with indirect DMA,
    the receiver one-hot chunk [128, n_obj] is rebuilt on-chip with a
    tensor_scalar is_equal against a free-dim iota (VectorE), the 4-layer
    relation MLP runs feature-major on the PE, and edge effects are
    aggregated to nodes with e_agg.T += e_chunk.T @ onehot_chunk into a
    pinned PSUM accumulator.
  - Partial e_agg is AllReduce-summed across the 8 cores; every core then
    runs the small object MLP on all 2048 nodes; host takes core 0's output.
"""

import os
import sys

import numpy as np

os.environ.setdefault("MYCRO_LOCAL_CACHE", "1")
for _p in ("/opt/trn_rl_repo",):
    if os.path.isdir(_p) and _p not in sys.path:
        sys.path.insert(0, _p)

import concourse.bacc as bacc
import concourse.bass as bass
import concourse.mybir as mybir
import concourse.tile as tile
from concourse.bass_utils import run_bass_kernel_spmd
from concourse.masks import make_identity

P = 128
F32 = mybir.dt.float32
F16 = mybir.dt.float16
I32 = mybir.dt.int32
I16 = mybir.dt.int16
AF = mybir.ActivationFunctionType
ALU = mybir.AluOpType

N_OBJ, N_REL = 2048, 32768
D_OBJ, D_REL, D_EFF = 64, 32, 64
H_REL, H_OBJ = 128, 128
D_OUT = 3
N_CORES = 8
E_PER_CORE = N_REL // N_CORES
N_CHUNKS = E_PER_CORE // P  # 32


def build(n_cores=N_CORES, e_per_core=E_PER_CORE, n_obj=N_OBJ,
          use_collective=True):
    EG = 512                  # edges per MLP group
    T = EG // P               # 128-edge chunks per group
    n_groups = e_per_core // EG
    n_chunks = e_per_core // P
    NQ = 512                  # node chunk (psum bank) for wide matmuls
    n_nq = n_obj // NQ

    nc = bacc.Bacc(
        "TRN2",
        target_bir_lowering=False,
        debug=False,
        enable_asserts=False,
        num_devices=n_cores,
    )

    idx = nc.dram_tensor("idx_c", [P, 2 * n_chunks], I32, kind="ExternalInput")
    raT = nc.dram_tensor("raT_c", [D_REL, e_per_core], F32, kind="ExternalInput")
    obj = nc.dram_tensor("obj", [n_obj, D_OBJ], F32, kind="ExternalInput")
    rm_w1 = nc.dram_tensor("rm_w1", [2 * D_OBJ + D_REL, H_REL], F32, kind="ExternalInput")
    rm_b1 = nc.dram_tensor("rm_b1", [H_REL], F32, kind="ExternalInput")
    rm_w2 = nc.dram_tensor("rm_w2", [H_REL, H_REL], F32, kind="ExternalInput")
    rm_b2 = nc.dram_tensor("rm_b2", [H_REL], F32, kind="ExternalInput")
    rm_w3 = nc.dram_tensor("rm_w3", [H_REL, H_REL], F32, kind="ExternalInput")
    rm_b3 = nc.dram_tensor("rm_b3", [H_REL], F32, kind="ExternalInput")
    rm_w4 = nc.dram_tensor("rm_w4", [H_REL, D_EFF], F32, kind="ExternalInput")
    rm_b4 = nc.dram_tensor("rm_b4", [D_EFF], F32, kind="ExternalInput")
    om_w1 = nc.dram_tensor("om_w1", [D_OBJ + D_EFF, H_OBJ], F32, kind="ExternalInput")
    om_b1 = nc.dram_tensor("om_b1", [H_OBJ], F32, kind="ExternalInput")
    om_w2 = nc.dram_tensor("om_w2", [H_OBJ, D_OUT], F32, kind="ExternalInput")
    om_b2 = nc.dram_tensor("om_b2", [D_OUT], F32, kind="ExternalInput")
    pT_d = nc.dram_tensor("pT", [D_OUT, n_obj], F32, kind="ExternalOutput")

    with tile.TileContext(nc) as tc:
        with (
            tc.tile_pool(name="const", bufs=1) as const,
            tc.tile_pool(name="stream", bufs=8) as sp,
            tc.tile_pool(name="gat", bufs=4) as gp,
            tc.tile_pool(name="ec", bufs=8) as ecp,
            tc.tile_pool(name="aggp", bufs=1, space="PSUM") as aggp,
            tc.tile_pool(name="psp", bufs=4, space="PSUM") as psp,
            tc.tile_pool(name="dram", bufs=1, space="DRAM") as dp,
        ):
            # ---- constants -------------------------------------------------
            ident32 = const.tile([P, P], F32)
            make_identity(nc, ident32[:])
            ident16 = const.tile([P, P], F16)
            make_identity(nc, ident16[:])

            iota_i = const.tile([P, n_obj], I16)
            nc.gpsimd.iota(iota_i[:], pattern=[[1, n_obj]], base=0, channel_multiplier=0)
            iota16 = const.tile([P, n_obj], F16)
            nc.vector.tensor_copy(iota16[:], iota_i[:])

            idx_sb = const.tile([P, 2 * n_chunks], I32)
            nc.sync.dma_start(idx_sb[:], idx[:, :])
            idxf32 = const.tile([P, n_chunks], F32)
            nc.vector.tensor_copy(idxf32[:], idx_sb[:, 0:n_chunks])

            w1ab = const.tile([P, H_REL], F32)
            nc.sync.dma_start(w1ab[:], rm_w1[0:P, :])
            w1c = const.tile([D_REL, H_REL], F32)
            nc.sync.dma_start(w1c[:], rm_w1[P : P + D_REL, :])
            w2 = const.tile([H_REL, H_REL], F32)
            nc.sync.dma_start(w2[:], rm_w2[:, :])
            w3 = const.tile([H_REL, H_REL], F32)
            nc.sync.dma_start(w3[:], rm_w3[:, :])
            w4 = const.tile([H_REL, D_EFF], F32)
            nc.sync.dma_start(w4[:], rm_w4[:, :])
            b1t = const.tile([H_REL, 1], F32)
            nc.sync.dma_start(b1t[:], rm_b1[:, None])
            b2t = const.tile([H_REL, 1], F32)
            nc.sync.dma_start(b2t[:], rm_b2[:, None])
            b3t = const.tile([H_REL, 1], F32)
            nc.sync.dma_start(b3t[:], rm_b3[:, None])
            b4t = const.tile([D_EFF, 1], F32)
            nc.sync.dma_start(b4t[:], rm_b4[:, None])
            ow1a = const.tile([D_OBJ, H_OBJ], F32)
            nc.sync.dma_start(ow1a[:], om_w1[0:D_OBJ, :])
            ow1b = const.tile([D_EFF, H_OBJ], F32)
            nc.sync.dma_start(ow1b[:], om_w1[D_OBJ : D_OBJ + D_EFF, :])
            ow2 = const.tile([H_OBJ, D_OUT], F32)
            nc.sync.dma_start(ow2[:], om_w2[:, :])
            ob1t = const.tile([H_OBJ, 1], F32)
            nc.sync.dma_start(ob1t[:], om_b1[:, None])
            ob2t = const.tile([D_OUT, 1], F32)
            nc.sync.dma_start(ob2t[:], om_b2[:, None])

            # obj.T in SBUF (for the node-model MLP)
            objT = const.tile([D_OBJ, n_obj], F32)
            for k in range(n_obj // P):
                ot = gp.tile([P, D_OBJ], F32, tag="objload")
                nc.sync.dma_start(ot[:], obj[k * P : (k + 1) * P, :])
                tp = psp.tile([D_OBJ, P], F32, tag="ps")
                nc.tensor.transpose(tp[:], ot[:], ident32[:])
                nc.scalar.copy(objT[:, k * P : (k + 1) * P], tp[:])

            # pinned accumulator: e_agg.T [64, n_obj] (4 PSUM banks)
            agg_ps = aggp.tile([D_EFF, n_obj], F32)

            # ---- edge phase ------------------------------------------------
            for g in range(n_groups):
                e0 = g * EG
                oht = []
                for t in range(T):
                    c = g * T + t
                    oh = sp.tile([P, n_obj], F16, tag="oh")
                    nc.vector.tensor_scalar(
                        oh[:], iota16[:], idxf32[:, c : c + 1], None,
                        op0=ALU.is_equal,
                    )
                    oht.append(oh)

                raTg = sp.tile([D_REL, EG], F32, tag="raT")
                nc.sync.dma_start(raTg[:], raT[:, e0 : e0 + EG])

                b1T = sp.tile([P, EG], F32, tag="b1T")
                for t in range(T):
                    c = g * T + t
                    orr_t = gp.tile([P, D_OBJ], F32, tag="gat")
                    nc.gpsimd.indirect_dma_start(
                        out=orr_t[:], out_offset=None, in_=obj[:, :],
                        in_offset=bass.IndirectOffsetOnAxis(
                            ap=idx_sb[:, c : c + 1], axis=0
                        ),
                    )
                    tp = psp.tile([D_OBJ, P], F32, tag="ps")
                    nc.tensor.transpose(tp[:], orr_t[:], ident32[:])
                    nc.scalar.copy(b1T[0:D_OBJ, t * P : (t + 1) * P], tp[:])

                    ors_t = gp.tile([P, D_OBJ], F32, tag="gat")
                    nc.gpsimd.indirect_dma_start(
                        out=ors_t[:], out_offset=None, in_=obj[:, :],
                        in_offset=bass.IndirectOffsetOnAxis(
                            ap=idx_sb[:, n_chunks + c : n_chunks + c + 1], axis=0
                        ),
                    )
                    tp2 = psp.tile([D_OBJ, P], F32, tag="ps")
                    nc.tensor.transpose(tp2[:], ors_t[:], ident32[:])
                    nc.scalar.copy(b1T[D_OBJ : 2 * D_OBJ, t * P : (t + 1) * P], tp2[:])

                # relation MLP, feature-major [features, EG]
                h1p = psp.tile([H_REL, EG], F32, tag="ps")
                nc.tensor.matmul(h1p[:], w1ab[:], b1T[:], start=True, stop=False)
                nc.tensor.matmul(h1p[:], w1c[:], raTg[:], start=False, stop=True)
                h1T = sp.tile([H_REL, EG], F32, tag="hT")
                nc.scalar.activation(h1T[:], h1p[:], AF.Relu, bias=b1t[:])

                h2p = psp.tile([H_REL, EG], F32, tag="ps")
                nc.tensor.matmul(h2p[:], w2[:], h1T[:], start=True, stop=True)
                h2T = sp.tile([H_REL, EG], F32, tag="hT")
                nc.scalar.activation(h2T[:], h2p[:], AF.Relu, bias=b2t[:])

                h3p = psp.tile([H_REL, EG], F32, tag="ps")
                nc.tensor.matmul(h3p[:], w3[:], h2T[:], start=True, stop=True)
                h3T = sp.tile([H_REL, EG], F32, tag="hT")
                nc.scalar.activation(h3T[:], h3p[:], AF.Relu, bias=b3t[:])

                h4p = psp.tile([D_EFF, EG], F32, tag="ps")
                nc.tensor.matmul(h4p[:], w4[:], h3T[:], start=True, stop=True)
                eT = sp.tile([D_EFF, EG], F16, tag="eT")
                nc.scalar.activation(eT[:], h4p[:], AF.Relu, bias=b4t[:])

                # aggregate: e_agg.T += e_chunk.T @ onehot_chunk
                for t in range(T):
                    ep = psp.tile([P, D_EFF], F16, tag="ps")
                    nc.tensor.transpose(
                        ep[:], eT[:, t * P : (t + 1) * P], ident16[:D_EFF, :D_EFF]
                    )
                    ec = ecp.tile([P, D_EFF], F16, tag="ec")
                    nc.scalar.copy(ec[:], ep[:])
                    first = g == 0 and t == 0
                    last = g == n_groups - 1 and t == T - 1
                    for q in range(n_obj // NQ):
                        nc.tensor.matmul(
                            agg_ps[:, q * NQ : (q + 1) * NQ],
                            ec[:],
                            oht[t][:, q * NQ : (q + 1) * NQ],
                            start=first,
                            stop=last,
                        )

            # ---- all-reduce e_agg across cores -----------------------------
            eagg_sb = const.tile([D_EFF, n_obj], F32)
            nc.scalar.copy(eagg_sb[:], agg_ps[:])
            cc_in = dp.tile([D_EFF, n_obj], F32)
            cc_out = dp.tile([D_EFF, n_obj], F32)
            nc.sync.dma_start(cc_in[:], eagg_sb[:])
            if use_collective:
                nc.gpsimd.collective_compute(
                    "AllReduce",
                    ALU.add,
                    replica_groups=[list(range(n_cores))],
                    ins=[cc_in.opt()],
                    outs=[cc_out.opt()],
                )
            else:
                nc.sync.dma_start(cc_out[:], cc_in[:])
            eaggT = const.tile([D_EFF, n_obj], F32)
            nc.sync.dma_start(eaggT[:], cc_out[:])

            # ---- node phase (object MLP) -----------------------------------
            pTt = const.tile([D_OUT, n_obj], F32)
            for q in range(n_nq):
                sl = slice(q * NQ, (q + 1) * NQ)
                cp = psp.tile([H_OBJ, NQ], F32, tag="ps")
                nc.tensor.matmul(cp[:], ow1a[:], objT[:, sl], start=True, stop=False)
                nc.tensor.matmul(cp[:], ow1b[:], eaggT[:, sl], start=False, stop=True)
                hT = sp.tile([H_OBJ, NQ], F32, tag="hT")
                nc.scalar.activation(hT[:], cp[:], AF.Relu, bias=ob1t[:])
                pp = psp.tile([D_OUT, NQ], F32, tag="ps")
                nc.tensor.matmul(pp[:], ow2[:], hT[:], start=True, stop=True)
                nc.scalar.activation(pTt[:, sl], pp[:], AF.Identity, bias=ob2t[:])
            nc.sync.dma_start(pT_d[:, :], pTt[:])

    nc.compile()
    return nc


_CACHE = {}
TRACE = False
_IOTA = np.arange(N_OBJ, dtype=np.float32)


def _get_nc():
    if "nc" not in _CACHE:
        _CACHE["nc"] = build()
    return _CACHE["nc"]


def _onehot_to_idx(a):
    """Exact index recovery from a one-hot float matrix via iota GEMV."""
    a = np.asarray(a)
    if a.dtype != np.float32:
        a = a.astype(np.float32)
    return (a @ _IOTA).astype(np.int32)


def kernel(**inputs):
    nc = _get_nc()
    f = lambda k: np.ascontiguousarray(np.asarray(inputs[k], dtype=np.float32))
    shared = {
        "obj": f("obj"),
        "rm_w1": f("rm_w1"), "rm_b1": f("rm_b1"),
        "rm_w2": f("rm_w2"), "rm_b2": f("rm_b2"),
        "rm_w3": f("rm_w3"), "rm_b3": f("rm_b3"),
        "rm_w4": f("rm_w4"), "rm_b4": f("rm_b4"),
        "om_w1": f("om_w1"), "om_b1": f("om_b1"),
        "om_w2": f("om_w2"), "om_b2": f("om_b2"),
    }
    recv = _onehot_to_idx(inputs["rr"])
    send = _onehot_to_idx(inputs["rs"])
    ra = np.asarray(inputs["ra"], dtype=np.float32)
    in_maps = []
    for c in range(N_CORES):
        sl = slice(c * E_PER_CORE, (c + 1) * E_PER_CORE)
        m = dict(shared)
        m["idx_c"] = np.ascontiguousarray(
            np.concatenate(
                [recv[sl].reshape(N_CHUNKS, P).T, send[sl].reshape(N_CHUNKS, P).T],
                axis=1,
            ),
            dtype=np.int32,
        )
        m["raT_c"] = np.ascontiguousarray(ra[sl].T)
        in_maps.append(m)
    res = run_bass_kernel_spmd(
        nc, in_maps, core_ids=list(range(N_CORES)), trace=TRACE
    )
    _CACHE["last_results"] = res
    return np.ascontiguousarray(res.results[0]["pT"].T)


# revision 6
# speedup vs baseline: 35.8927x; 2.1902x over previous
"""InteractionNetwork (GNN message passing) Bass kernel for 8 Trainium2 cores.

Strategy (edge-sharded, per sharding hint):
  - The rr/rs one-hot matrices are a dense encoding of receiver/sender index
    vectors. The host losslessly re-encodes them as indices (exact GEMV
    against an iota vector), so each call ships ~3 MB instead of ~540 MB
    through the PJRT tunnel.
  - Edges are sharded across 8 cores (4096 each). On device, per 128-edge
    chunk: receiver/sender node features are gathered with indirect DMA,
    the receiver one-hot chunk [128, n_obj] is rebuilt on-chip with a
    tensor_scalar is_equal against a free-dim iota (VectorE), the 4-layer
    relation MLP runs feature-major on the PE, and edge effects are
    aggregated to nodes with e_agg.T += e_chunk.T @ onehot_chunk into a
    pinned PSUM accumulator.
  - Partial e_agg is AllReduce-summed across the 8 cores; every core then
    runs the small object MLP on all 2048 nodes; host takes core 0's output.
  - The axon tunnel is latency-bound (~70 ms/sync, ~50 MB/s), so the host
    caches the jitted executable across calls, sends the replicated
    weights/obj with a replicated sharding (1x wire bytes, not 8x), ships
    obj/ra/idx as f16, issues all transfers asynchronously (overlapped with
    the index-extraction GEMVs), and syncs exactly once on the output.
"""

import os
import sys

import numpy as np

os.environ.setdefault("MYCRO_LOCAL_CACHE", "1")
for _p in ("/opt/trn_rl_repo",):
    if os.path.isdir(_p) and _p not in sys.path:
        sys.path.insert(0, _p)

import concourse.bacc as bacc
import concourse.bass as bass
import concourse.mybir as mybir
import concourse.tile as tile
from concourse._compat import axon_active
from concourse.bass_utils import run_bass_kernel_spmd
from concourse.masks import make_identity

P = 128
F32 = mybir.dt.float32
F16 = mybir.dt.float16
I32 = mybir.dt.int32
I16 = mybir.dt.int16
AF = mybir.ActivationFunctionType
ALU = mybir.AluOpType

N_OBJ, N_REL = 2048, 32768
D_OBJ, D_REL, D_EFF = 64, 32, 64
H_REL, H_OBJ = 128, 128
D_OUT = 3
N_CORES = 8
E_PER_CORE = N_REL // N_CORES
N_CHUNKS = E_PER_CORE // P  # 32

# inputs sharded along the edge dimension; everything else replicated
_SHARDED_INPUTS = {"idx_c", "raT_c"}


def build(n_cores=N_CORES, e_per_core=E_PER_CORE, n_obj=N_OBJ,
          use_collective=True):
    EG = 512                  # edges per MLP group
    T = EG // P               # 128-edge chunks per group
    n_groups = e_per_core // EG
    n_chunks = e_per_core // P
    NQ = 512                  # node chunk (psum bank) for wide matmuls
    n_nq = n_obj // NQ

    nc = bacc.Bacc(
        "TRN2",
        target_bir_lowering=False,
        debug=False,
        enable_asserts=False,
        num_devices=n_cores,
    )

    idx = nc.dram_tensor("idx_c", [P, 2 * n_chunks], F16, kind="ExternalInput")
    raT = nc.dram_tensor("raT_c", [D_REL, e_per_core], F16, kind="ExternalInput")
    obj = nc.dram_tensor("obj", [n_obj, D_OBJ], F16, kind="ExternalInput")
    rm_w1 = nc.dram_tensor("rm_w1", [2 * D_OBJ + D_REL, H_REL], F32, kind="ExternalInput")
    rm_b1 = nc.dram_tensor("rm_b1", [H_REL], F32, kind="ExternalInput")
    rm_w2 = nc.dram_tensor("rm_w2", [H_REL, H_REL], F32, kind="ExternalInput")
    rm_b2 = nc.dram_tensor("rm_b2", [H_REL], F32, kind="ExternalInput")
    rm_w3 = nc.dram_tensor("rm_w3", [H_REL, H_REL], F32, kind="ExternalInput")
    rm_b3 = nc.dram_tensor("rm_b3", [H_REL], F32, kind="ExternalInput")
    rm_w4 = nc.dram_tensor("rm_w4", [H_REL, D_EFF], F32, kind="ExternalInput")
    rm_b4 = nc.dram_tensor("rm_b4", [D_EFF], F32, kind="ExternalInput")
    om_w1 = nc.dram_tensor("om_w1", [D_OBJ + D_EFF, H_OBJ], F32, kind="ExternalInput")
    om_b1 = nc.dram_tensor("om_b1", [H_OBJ], F32, kind="ExternalInput")
    om_w2 = nc.dram_tensor("om_w2", [H_OBJ, D_OUT], F32, kind="ExternalInput")
    om_b2 = nc.dram_tensor("om_b2", [D_OUT], F32, kind="ExternalInput")
    pT_d = nc.dram_tensor("pT", [D_OUT, n_obj], F32, kind="ExternalOutput")

    with tile.TileContext(nc) as tc:
        with (
            tc.tile_pool(name="const", bufs=1) as const,
            tc.tile_pool(name="stream", bufs=8) as sp,
            tc.tile_pool(name="gat", bufs=4) as gp,
            tc.tile_pool(name="ec", bufs=8) as ecp,
            tc.tile_pool(name="aggp", bufs=1, space="PSUM") as aggp,
            tc.tile_pool(name="psp", bufs=4, space="PSUM") as psp,
            tc.tile_pool(name="dram", bufs=1, space="DRAM") as dp,
        ):
            # ---- constants -------------------------------------------------
            ident32 = const.tile([P, P], F32)
            make_identity(nc, ident32[:])
            ident16 = const.tile([P, P], F16)
            make_identity(nc, ident16[:])

            iota_i = const.tile([P, n_obj], I16)
            nc.gpsimd.iota(iota_i[:], pattern=[[1, n_obj]], base=0, channel_multiplier=0)
            iota16 = const.tile([P, n_obj], F16)
            nc.vector.tensor_copy(iota16[:], iota_i[:])

            idx_sb16 = const.tile([P, 2 * n_chunks], F16)
            nc.sync.dma_start(idx_sb16[:], idx[:, :])
            idx_sb = const.tile([P, 2 * n_chunks], I32)
            nc.vector.tensor_copy(idx_sb[:], idx_sb16[:])
            idxf32 = const.tile([P, n_chunks], F32)
            nc.vector.tensor_copy(idxf32[:], idx_sb16[:, 0:n_chunks])

            w1ab = const.tile([P, H_REL], F32)
            nc.sync.dma_start(w1ab[:], rm_w1[0:P, :])
            w1c = const.tile([D_REL, H_REL], F32)
            nc.sync.dma_start(w1c[:], rm_w1[P : P + D_REL, :])
            w2 = const.tile([H_REL, H_REL], F32)
            nc.sync.dma_start(w2[:], rm_w2[:, :])
            w3 = const.tile([H_REL, H_REL], F32)
            nc.sync.dma_start(w3[:], rm_w3[:, :])
            w4 = const.tile([H_REL, D_EFF], F32)
            nc.sync.dma_start(w4[:], rm_w4[:, :])
            b1t = const.tile([H_REL, 1], F32)
            nc.sync.dma_start(b1t[:], rm_b1[:, None])
            b2t = const.tile([H_REL, 1], F32)
            nc.sync.dma_start(b2t[:], rm_b2[:, None])
            b3t = const.tile([H_REL, 1], F32)
            nc.sync.dma_start(b3t[:], rm_b3[:, None])
            b4t = const.tile([D_EFF, 1], F32)
            nc.sync.dma_start(b4t[:], rm_b4[:, None])
            ow1a = const.tile([D_OBJ, H_OBJ], F32)
            nc.sync.dma_start(ow1a[:], om_w1[0:D_OBJ, :])
            ow1b = const.tile([D_EFF, H_OBJ], F32)
            nc.sync.dma_start(ow1b[:], om_w1[D_OBJ : D_OBJ + D_EFF, :])
            ow2 = const.tile([H_OBJ, D_OUT], F32)
            nc.sync.dma_start(ow2[:], om_w2[:, :])
            ob1t = const.tile([H_OBJ, 1], F32)
            nc.sync.dma_start(ob1t[:], om_b1[:, None])
            ob2t = const.tile([D_OUT, 1], F32)
            nc.sync.dma_start(ob2t[:], om_b2[:, None])

            # obj.T in SBUF (for the node-model MLP), f16 -> f32
            objT = const.tile([D_OBJ, n_obj], F32)
            for k in range(n_obj // P):
                ot = gp.tile([P, D_OBJ], F16, tag="objload")
                nc.sync.dma_start(ot[:], obj[k * P : (k + 1) * P, :])
                tp = psp.tile([D_OBJ, P], F16, tag="ps")
                nc.tensor.transpose(tp[:], ot[:], ident16[:])
                nc.scalar.copy(objT[:, k * P : (k + 1) * P], tp[:])

            # pinned accumulator: e_agg.T [64, n_obj] (4 PSUM banks)
            agg_ps = aggp.tile([D_EFF, n_obj], F32)

            # ---- edge phase ------------------------------------------------
            for g in range(n_groups):
                e0 = g * EG
                oht = []
                for t in range(T):
                    c = g * T + t
                    oh = sp.tile([P, n_obj], F16, tag="oh")
                    nc.vector.tensor_scalar(
                        oh[:], iota16[:], idxf32[:, c : c + 1], None,
                        op0=ALU.is_equal,
                    )
                    oht.append(oh)

                raTg16 = sp.tile([D_REL, EG], F16, tag="raT16")
                nc.sync.dma_start(raTg16[:], raT[:, e0 : e0 + EG])
                raTg = sp.tile([D_REL, EG], F32, tag="raT")
                nc.vector.tensor_copy(raTg[:], raTg16[:])

                b1T = sp.tile([P, EG], F32, tag="b1T")
                for t in range(T):
                    c = g * T + t
                    orr_t = gp.tile([P, D_OBJ], F16, tag="gat")
                    nc.gpsimd.indirect_dma_start(
                        out=orr_t[:], out_offset=None, in_=obj[:, :],
                        in_offset=bass.IndirectOffsetOnAxis(
                            ap=idx_sb[:, c : c + 1], axis=0
                        ),
                    )
                    tp = psp.tile([D_OBJ, P], F16, tag="ps")
                    nc.tensor.transpose(tp[:], orr_t[:], ident16[:])
                    nc.scalar.copy(b1T[0:D_OBJ, t * P : (t + 1) * P], tp[:])

                    ors_t = gp.tile([P, D_OBJ], F16, tag="gat")
                    nc.gpsimd.indirect_dma_start(
                        out=ors_t[:], out_offset=None, in_=obj[:, :],
                        in_offset=bass.IndirectOffsetOnAxis(
                            ap=idx_sb[:, n_chunks + c : n_chunks + c + 1], axis=0
                        ),
                    )
                    tp2 = psp.tile([D_OBJ, P], F16, tag="ps")
                    nc.tensor.transpose(tp2[:], ors_t[:], ident16[:])
                    nc.scalar.copy(b1T[D_OBJ : 2 * D_OBJ, t * P : (t + 1) * P], tp2[:])

                # relation MLP, feature-major [features, EG]
                h1p = psp.tile([H_REL, EG], F32, tag="ps")
                nc.tensor.matmul(h1p[:], w1ab[:], b1T[:], start=True, stop=False)
                nc.tensor.matmul(h1p[:], w1c[:], raTg[:], start=False, stop=True)
                h1T = sp.tile([H_REL, EG], F32, tag="hT")
                nc.scalar.activation(h1T[:], h1p[:], AF.Relu, bias=b1t[:])

                h2p = psp.tile([H_REL, EG], F32, tag="ps")
                nc.tensor.matmul(h2p[:], w2[:], h1T[:], start=True, stop=True)
                h2T = sp.tile([H_REL, EG], F32, tag="hT")
                nc.scalar.activation(h2T[:], h2p[:], AF.Relu, bias=b2t[:])

                h3p = psp.tile([H_REL, EG], F32, tag="ps")
                nc.tensor.matmul(h3p[:], w3[:], h2T[:], start=True, stop=True)
                h3T = sp.tile([H_REL, EG], F32, tag="hT")
                nc.scalar.activation(h3T[:], h3p[:], AF.Relu, bias=b3t[:])

                h4p = psp.tile([D_EFF, EG], F32, tag="ps")
                nc.tensor.matmul(h4p[:], w4[:], h3T[:], start=True, stop=True)
                eT = sp.tile([D_EFF, EG], F16, tag="eT")
                nc.scalar.activation(eT[:], h4p[:], AF.Relu, bias=b4t[:])

                # aggregate: e_agg.T += e_chunk.T @ onehot_chunk
                for t in range(T):
                    ep = psp.tile([P, D_EFF], F16, tag="ps")
                    nc.tensor.transpose(
                        ep[:], eT[:, t * P : (t + 1) * P], ident16[:D_EFF, :D_EFF]
                    )
                    ec = ecp.tile([P, D_EFF], F16, tag="ec")
                    nc.scalar.copy(ec[:], ep[:])
                    first = g == 0 and t == 0
                    last = g == n_groups - 1 and t == T - 1
                    for q in range(n_obj // NQ):
                        nc.tensor.matmul(
                            agg_ps[:, q * NQ : (q + 1) * NQ],
                            ec[:],
                            oht[t][:, q * NQ : (q + 1) * NQ],
                            start=first,
                            stop=last,
                        )

            # ---- all-reduce e_agg across cores -----------------------------
            eagg_sb = const.tile([D_EFF, n_obj], F32)
            nc.scalar.copy(eagg_sb[:], agg_ps[:])
            cc_in = dp.tile([D_EFF, n_obj], F32)
            cc_out = dp.tile([D_EFF, n_obj], F32)
            nc.sync.dma_start(cc_in[:], eagg_sb[:])
            if use_collective:
                nc.gpsimd.collective_compute(
                    "AllReduce",
                    ALU.add,
                    replica_groups=[list(range(n_cores))],
                    ins=[cc_in.opt()],
                    outs=[cc_out.opt()],
                )
            else:
                nc.sync.dma_start(cc_out[:], cc_in[:])
            eaggT = const.tile([D_EFF, n_obj], F32)
            nc.sync.dma_start(eaggT[:], cc_out[:])

            # ---- node phase (object MLP) -----------------------------------
            pTt = const.tile([D_OUT, n_obj], F32)
            for q in range(n_nq):
                sl = slice(q * NQ, (q + 1) * NQ)
                cp = psp.tile([H_OBJ, NQ], F32, tag="ps")
                nc.tensor.matmul(cp[:], ow1a[:], objT[:, sl], start=True, stop=False)
                nc.tensor.matmul(cp[:], ow1b[:], eaggT[:, sl], start=False, stop=True)
                hT = sp.tile([H_OBJ, NQ], F32, tag="hT")
                nc.scalar.activation(hT[:], cp[:], AF.Relu, bias=ob1t[:])
                pp = psp.tile([D_OUT, NQ], F32, tag="ps")
                nc.tensor.matmul(pp[:], ow2[:], hT[:], start=True, stop=True)
                nc.scalar.activation(pTt[:, sl], pp[:], AF.Identity, bias=ob2t[:])
            nc.sync.dma_start(pT_d[:, :], pTt[:])

    nc.compile()
    return nc


_CACHE = {}
TRACE = False
_IOTA = np.arange(N_OBJ, dtype=np.float32)


def _get_nc():
    if "nc" not in _CACHE:
        _CACHE["nc"] = build()
    return _CACHE["nc"]


def _onehot_to_idx(a):
    """Exact index recovery from a one-hot float matrix via iota GEMV."""
    a = np.asarray(a)
    if a.dtype != np.float32:
        a = a.astype(np.float32)
    return a @ _IOTA


def _idx_blocks(v):
    """[N_REL] float indices -> [N_CORES*P, N_CHUNKS] f16, per-core chunk-major."""
    # per core: [E_PER_CORE] -> (N_CHUNKS, P) -> T -> [P, N_CHUNKS]
    return np.ascontiguousarray(
        np.transpose(v.reshape(N_CORES, N_CHUNKS, P), (0, 2, 1)).reshape(
            N_CORES * P, N_CHUNKS
        ),
        dtype=np.float16,
    )


def _get_runner():
    """Build (once) a cached jitted shard_map executable over the Bass NEFF."""
    if "runner" in _CACHE:
        return _CACHE["runner"]

    import jax
    from jax.experimental.shard_map import shard_map
    from jax.sharding import Mesh, NamedSharding, PartitionSpec

    from concourse.bass2jax import (
        _bass_exec_p,
        install_neuronx_cc_hook,
        partition_id_tensor,
    )

    nc = _get_nc()
    install_neuronx_cc_hook()
    partition_name = nc.partition_id_tensor.name if nc.partition_id_tensor else None
    in_names, out_names, out_avals, zero_outs = [], [], [], []
    for alloc in nc.m.functions[0].allocations:
        if not isinstance(alloc, mybir.MemoryLocationSet):
            continue
        name = alloc.memorylocations[0].name
        if alloc.kind == "ExternalInput":
            if name != partition_name:
                in_names.append(name)
        elif alloc.kind == "ExternalOutput":
            out_names.append(name)
            shape = tuple(alloc.tensor_shape)
            dtype = mybir.dt.np(alloc.dtype)
            out_avals.append(jax.core.ShapedArray(shape, dtype))
            zero_outs.append(np.zeros((N_CORES * shape[0], *shape[1:]), dtype))
    n_params = len(in_names)
    n_outs = len(out_avals)
    param_names = list(in_names)
    in_names = in_names + out_names
    if partition_name is not None:
        in_names.append(partition_name)

    def _body(*args):
        operands = list(args)
        if partition_name is not None:
            operands.append(partition_id_tensor())
        outs = _bass_exec_p.bind(
            *operands,
            out_avals=tuple(out_avals),
            in_names=tuple(in_names),
            out_names=tuple(out_names),
            lowering_input_output_aliases=(),
            sim_require_finite=True,
            sim_require_nnan=True,
            nc=nc,
        )
        return tuple(outs)

    devices = jax.devices()[:N_CORES]
    mesh = Mesh(np.asarray(devices), ("core",))
    shard = NamedSharding(mesh, PartitionSpec("core"))
    repl = NamedSharding(mesh, PartitionSpec())
    param_specs = tuple(
        PartitionSpec("core") if n in _SHARDED_INPUTS else PartitionSpec()
        for n in param_names
    )
    in_specs = param_specs + (PartitionSpec("core"),) * n_outs
    out_specs = (PartitionSpec("core"),) * n_outs
    sharded = jax.jit(
        shard_map(_body, mesh=mesh, in_specs=in_specs, out_specs=out_specs,
                  check_rep=False),
        donate_argnums=tuple(range(n_params, n_params + n_outs)),
        keep_unused=True,
    )
    runner = dict(
        jax=jax, sharded=sharded, param_names=param_names,
        zero_outs=zero_outs, shard=shard, repl=repl, out_names=out_names,
    )
    _CACHE["runner"] = runner
    return runner


def kernel(**inputs):
    if not axon_active():
        return _kernel_fallback(**inputs)
    try:
        return _kernel_fast(**inputs)
    except Exception:
        _CACHE.pop("runner", None)
        return _kernel_fallback(**inputs)


def _kernel_fast(**inputs):
    r = _get_runner()
    jax, sharded = r["jax"], r["sharded"]
    shard, repl = r["shard"], r["repl"]

    # 1) issue the cheap, index-independent uploads first (all async)
    devs = {}
    devs["obj"] = jax.device_put(
        np.asarray(inputs["obj"]).astype(np.float16), repl)
    for k in ("rm_w1", "rm_b1", "rm_w2", "rm_b2", "rm_w3", "rm_b3",
              "rm_w4", "rm_b4", "om_w1", "om_b1", "om_w2", "om_b2"):
        a = np.asarray(inputs[k])
        if a.dtype != np.float32:
            a = a.astype(np.float32)
        devs[k] = jax.device_put(a, repl)
    ra = np.asarray(inputs["ra"])
    raT = np.ascontiguousarray(
        np.transpose(ra.reshape(N_CORES, E_PER_CORE, D_REL), (0, 2, 1)).reshape(
            N_CORES * D_REL, E_PER_CORE
        ),
        dtype=np.float16,
    )
    devs["raT_c"] = jax.device_put(raT, shard)
    zeros = [jax.device_put(z, shard) for z in r["zero_outs"]]

    # 2) index extraction overlaps with the transfers above
    recv = _onehot_to_idx(inputs["rr"])
    send = _onehot_to_idx(inputs["rs"])
    idx = np.concatenate([_idx_blocks(recv), _idx_blocks(send)], axis=1)
    devs["idx_c"] = jax.device_put(idx, shard)

    # 3) dispatch + single sync on the output
    out_arrs = sharded(*[devs[n] for n in r["param_names"]], *zeros)
    _CACHE["last_results"] = None
    pT_all = np.asarray(out_arrs[r["out_names"].index("pT")])
    return np.ascontiguousarray(pT_all[0:D_OUT, :].T)


def _kernel_fallback(**inputs):
    """Non-axon path: run through bass_utils with per-core input maps."""
    nc = _get_nc()
    f = lambda k: np.ascontiguousarray(np.asarray(inputs[k], dtype=np.float32))
    shared = {
        "obj": np.asarray(inputs["obj"]).astype(np.float16),
        "rm_w1": f("rm_w1"), "rm_b1": f("rm_b1"),
        "rm_w2": f("rm_w2"), "rm_b2": f("rm_b2"),
        "rm_w3": f("rm_w3"), "rm_b3": f("rm_b3"),
        "rm_w4": f("rm_w4"), "rm_b4": f("rm_b4"),
        "om_w1": f("om_w1"), "om_b1": f("om_b1"),
        "om_w2": f("om_w2"), "om_b2": f("om_b2"),
    }
    recv = _onehot_to_idx(inputs["rr"])
    send = _onehot_to_idx(inputs["rs"])
    idx = np.concatenate([_idx_blocks(recv), _idx_blocks(send)], axis=1)
    ra = np.asarray(inputs["ra"])
    in_maps = []
    for c in range(N_CORES):
        sl = slice(c * E_PER_CORE, (c + 1) * E_PER_CORE)
        m = dict(shared)
        m["idx_c"] = np.ascontiguousarray(idx[c * P : (c + 1) * P, :])
        m["raT_c"] = np.ascontiguousarray(ra[sl].T, dtype=np.float16)
        in_maps.append(m)
    res = run_bass_kernel_spmd(
        nc, in_maps, core_ids=list(range(N_CORES)), trace=TRACE
    )
    _CACHE["last_results"] = res
    return np.ascontiguousarray(res.results[0]["pT"].T)


# revision 13
# speedup vs baseline: 50.0859x; 1.3954x over previous
"""InteractionNetwork (GNN message passing) Bass kernel for 8 Trainium2 cores.

Strategy (edge-sharded, per sharding hint):
  - The rr/rs one-hot matrices are a dense encoding of receiver/sender index
    vectors. The host losslessly re-encodes them as indices (exact GEMV
    against an iota vector), so each call ships ~3 MB instead of ~540 MB
    through the PJRT tunnel.
  - Edges are sharded across 8 cores (4096 each). On device, per 128-edge
    chunk: receiver/sender node features are gathered with indirect DMA,
    the receiver one-hot chunk [128, n_obj] is rebuilt on-chip with a
    tensor_scalar is_equal against a free-dim iota (VectorE), the 4-layer
    relation MLP runs feature-major on the PE, and edge effects are
    aggregated to nodes with e_agg.T += e_chunk.T @ onehot_chunk into a
    pinned PSUM accumulator.
  - Partial e_agg is AllReduce-summed across the 8 cores; every core then
    runs the small object MLP on all 2048 nodes; host takes core 0's output.
  - The axon tunnel is latency-bound (~70 ms/sync, ~50 MB/s), so the host
    caches the jitted executable across calls, sends the replicated
    weights/obj with a replicated sharding (1x wire bytes, not 8x), ships
    obj/ra/idx as f16, issues all transfers asynchronously (overlapped with
    the index-extraction GEMVs), and syncs exactly once on the output.
"""

import os
import sys

import numpy as np

os.environ.setdefault("MYCRO_LOCAL_CACHE", "1")
for _p in ("/opt/trn_rl_repo",):
    if os.path.isdir(_p) and _p not in sys.path:
        sys.path.insert(0, _p)

import concourse.bacc as bacc
import concourse.bass as bass
import concourse.mybir as mybir
import concourse.tile as tile
from concourse._compat import axon_active
from concourse.bass_utils import run_bass_kernel_spmd
from concourse.masks import make_identity

P = 128
F32 = mybir.dt.float32
F16 = mybir.dt.float16
I32 = mybir.dt.int32
I16 = mybir.dt.int16
AF = mybir.ActivationFunctionType
ALU = mybir.AluOpType

N_OBJ, N_REL = 2048, 32768
D_OBJ, D_REL, D_EFF = 64, 32, 64
H_REL, H_OBJ = 128, 128
D_OUT = 3
N_CORES = 8
E_PER_CORE = N_REL // N_CORES
N_CHUNKS = E_PER_CORE // P  # 32

# inputs sharded along the edge dimension; everything else replicated
_SHARDED_INPUTS = {"idx_c", "raT_c"}

# all small weight/bias tensors travel as one packed f32 blob (one RPC)
_WPACK_LAYOUT = [
    ("rm_w1", (2 * D_OBJ + D_REL, H_REL)),
    ("rm_w2", (H_REL, H_REL)),
    ("rm_w3", (H_REL, H_REL)),
    ("rm_w4", (H_REL, D_EFF)),
    ("om_w1", (D_OBJ + D_EFF, H_OBJ)),
    ("om_w2", (H_OBJ, D_OUT)),
    ("rm_b1", (H_REL,)),
    ("rm_b2", (H_REL,)),
    ("rm_b3", (H_REL,)),
    ("rm_b4", (D_EFF,)),
    ("om_b1", (H_OBJ,)),
    ("om_b2", (D_OUT,)),
]
_WPACK_OFF = {}
_o = 0
for _n, _s in _WPACK_LAYOUT:
    _WPACK_OFF[_n] = _o
    _o += int(np.prod(_s))
_WPACK_SIZE = _o


def _pack_weights(inputs):
    out = np.empty(_WPACK_SIZE, np.float32)
    for n, s in _WPACK_LAYOUT:
        a = np.asarray(inputs[n])
        if a.dtype != np.float32:
            a = a.astype(np.float32)
        out[_WPACK_OFF[n] : _WPACK_OFF[n] + a.size] = a.ravel()
    return out


def build(n_cores=N_CORES, e_per_core=E_PER_CORE, n_obj=N_OBJ,
          use_collective=True):
    EG = 512                  # edges per MLP group
    T = EG // P               # 128-edge chunks per group
    n_groups = e_per_core // EG
    n_chunks = e_per_core // P
    NQ = 512                  # node chunk (psum bank) for wide matmuls
    n_nq = n_obj // NQ

    nc = bacc.Bacc(
        "TRN2",
        target_bir_lowering=False,
        debug=False,
        enable_asserts=False,
        num_devices=n_cores,
    )

    idx = nc.dram_tensor("idx_c", [P, 2 * n_chunks], F16, kind="ExternalInput")
    raT = nc.dram_tensor("raT_c", [D_REL, e_per_core], F16, kind="ExternalInput")
    obj = nc.dram_tensor("obj", [n_obj, D_OBJ], F16, kind="ExternalInput")
    wpack = nc.dram_tensor("wpack", [_WPACK_SIZE], F32, kind="ExternalInput")
    pT_d = nc.dram_tensor("pT", [D_OUT, n_obj], F32, kind="ExternalOutput")

    def wview(name, r0, r1):
        """2-D AP over wpack for rows [r0:r1) of packed tensor `name`."""
        shape = dict(_WPACK_LAYOUT)[name]
        cols = shape[1] if len(shape) == 2 else 1
        o = _WPACK_OFF[name] + r0 * cols
        return wpack[o : o + (r1 - r0) * cols].rearrange(
            "(a b) -> a b", a=r1 - r0, b=cols
        )

    with tile.TileContext(nc) as tc:
        with (
            tc.tile_pool(name="const", bufs=1) as const,
            tc.tile_pool(name="stream", bufs=8) as sp,
            tc.tile_pool(name="gat", bufs=4) as gp,
            tc.tile_pool(name="ec", bufs=8) as ecp,
            tc.tile_pool(name="aggp", bufs=1, space="PSUM") as aggp,
            tc.tile_pool(name="psp", bufs=4, space="PSUM") as psp,
            tc.tile_pool(name="dram", bufs=1, space="DRAM") as dp,
        ):
            # ---- constants -------------------------------------------------
            ident32 = const.tile([P, P], F32)
            make_identity(nc, ident32[:])
            ident16 = const.tile([P, P], F16)
            make_identity(nc, ident16[:])

            iota_i = const.tile([P, n_obj], I16)
            nc.gpsimd.iota(iota_i[:], pattern=[[1, n_obj]], base=0, channel_multiplier=0)
            iota16 = const.tile([P, n_obj], F16)
            nc.vector.tensor_copy(iota16[:], iota_i[:])

            idx_sb16 = const.tile([P, 2 * n_chunks], F16)
            nc.sync.dma_start(idx_sb16[:], idx[:, :])
            idx_sb = const.tile([P, 2 * n_chunks], I32)
            nc.vector.tensor_copy(idx_sb[:], idx_sb16[:])
            idxf32 = const.tile([P, n_chunks], F32)
            nc.vector.tensor_copy(idxf32[:], idx_sb16[:, 0:n_chunks])

            w1ab = const.tile([P, H_REL], F32)
            nc.sync.dma_start(w1ab[:], wview("rm_w1", 0, P))
            w1c = const.tile([D_REL, H_REL], F32)
            nc.sync.dma_start(w1c[:], wview("rm_w1", P, P + D_REL))
            w2 = const.tile([H_REL, H_REL], F32)
            nc.sync.dma_start(w2[:], wview("rm_w2", 0, H_REL))
            w3 = const.tile([H_REL, H_REL], F32)
            nc.sync.dma_start(w3[:], wview("rm_w3", 0, H_REL))
            w4 = const.tile([H_REL, D_EFF], F32)
            nc.sync.dma_start(w4[:], wview("rm_w4", 0, H_REL))
            b1t = const.tile([H_REL, 1], F32)
            nc.sync.dma_start(b1t[:], wview("rm_b1", 0, H_REL))
            b2t = const.tile([H_REL, 1], F32)
            nc.sync.dma_start(b2t[:], wview("rm_b2", 0, H_REL))
            b3t = const.tile([H_REL, 1], F32)
            nc.sync.dma_start(b3t[:], wview("rm_b3", 0, H_REL))
            b4t = const.tile([D_EFF, 1], F32)
            nc.sync.dma_start(b4t[:], wview("rm_b4", 0, D_EFF))
            ow1a = const.tile([D_OBJ, H_OBJ], F32)
            nc.sync.dma_start(ow1a[:], wview("om_w1", 0, D_OBJ))
            ow1b = const.tile([D_EFF, H_OBJ], F32)
            nc.sync.dma_start(ow1b[:], wview("om_w1", D_OBJ, D_OBJ + D_EFF))
            ow2 = const.tile([H_OBJ, D_OUT], F32)
            nc.sync.dma_start(ow2[:], wview("om_w2", 0, H_OBJ))
            ob1t = const.tile([H_OBJ, 1], F32)
            nc.sync.dma_start(ob1t[:], wview("om_b1", 0, H_OBJ))
            ob2t = const.tile([D_OUT, 1], F32)
            nc.sync.dma_start(ob2t[:], wview("om_b2", 0, D_OUT))

            # obj.T in SBUF (for the node-model MLP), f16 -> f32
            objT = const.tile([D_OBJ, n_obj], F32)
            for k in range(n_obj // P):
                ot = gp.tile([P, D_OBJ], F16, tag="objload")
                nc.sync.dma_start(ot[:], obj[k * P : (k + 1) * P, :])
                tp = psp.tile([D_OBJ, P], F16, tag="ps")
                nc.tensor.transpose(tp[:], ot[:], ident16[:])
                nc.scalar.copy(objT[:, k * P : (k + 1) * P], tp[:])

            # pinned accumulator: e_agg.T [64, n_obj] (4 PSUM banks)
            agg_ps = aggp.tile([D_EFF, n_obj], F32)

            # ---- edge phase ------------------------------------------------
            for g in range(n_groups):
                e0 = g * EG
                oht = []
                for t in range(T):
                    c = g * T + t
                    oh = sp.tile([P, n_obj], F16, tag="oh")
                    nc.vector.tensor_scalar(
                        oh[:], iota16[:], idxf32[:, c : c + 1], None,
                        op0=ALU.is_equal,
                    )
                    oht.append(oh)

                raTg16 = sp.tile([D_REL, EG], F16, tag="raT16")
                nc.sync.dma_start(raTg16[:], raT[:, e0 : e0 + EG])
                raTg = sp.tile([D_REL, EG], F32, tag="raT")
                nc.vector.tensor_copy(raTg[:], raTg16[:])

                b1T = sp.tile([P, EG], F32, tag="b1T")
                for t in range(T):
                    c = g * T + t
                    orr_t = gp.tile([P, D_OBJ], F16, tag="gat")
                    nc.gpsimd.indirect_dma_start(
                        out=orr_t[:], out_offset=None, in_=obj[:, :],
                        in_offset=bass.IndirectOffsetOnAxis(
                            ap=idx_sb[:, c : c + 1], axis=0
                        ),
                    )
                    tp = psp.tile([D_OBJ, P], F16, tag="ps")
                    nc.tensor.transpose(tp[:], orr_t[:], ident16[:])
                    nc.scalar.copy(b1T[0:D_OBJ, t * P : (t + 1) * P], tp[:])

                    ors_t = gp.tile([P, D_OBJ], F16, tag="gat")
                    nc.gpsimd.indirect_dma_start(
                        out=ors_t[:], out_offset=None, in_=obj[:, :],
                        in_offset=bass.IndirectOffsetOnAxis(
                            ap=idx_sb[:, n_chunks + c : n_chunks + c + 1], axis=0
                        ),
                    )
                    tp2 = psp.tile([D_OBJ, P], F16, tag="ps")
                    nc.tensor.transpose(tp2[:], ors_t[:], ident16[:])
                    nc.scalar.copy(b1T[D_OBJ : 2 * D_OBJ, t * P : (t + 1) * P], tp2[:])

                # relation MLP, feature-major [features, EG]
                h1p = psp.tile([H_REL, EG], F32, tag="ps")
                nc.tensor.matmul(h1p[:], w1ab[:], b1T[:], start=True, stop=False)
                nc.tensor.matmul(h1p[:], w1c[:], raTg[:], start=False, stop=True)
                h1T = sp.tile([H_REL, EG], F32, tag="hT")
                nc.scalar.activation(h1T[:], h1p[:], AF.Relu, bias=b1t[:])

                h2p = psp.tile([H_REL, EG], F32, tag="ps")
                nc.tensor.matmul(h2p[:], w2[:], h1T[:], start=True, stop=True)
                h2T = sp.tile([H_REL, EG], F32, tag="hT")
                nc.scalar.activation(h2T[:], h2p[:], AF.Relu, bias=b2t[:])

                h3p = psp.tile([H_REL, EG], F32, tag="ps")
                nc.tensor.matmul(h3p[:], w3[:], h2T[:], start=True, stop=True)
                h3T = sp.tile([H_REL, EG], F32, tag="hT")
                nc.scalar.activation(h3T[:], h3p[:], AF.Relu, bias=b3t[:])

                h4p = psp.tile([D_EFF, EG], F32, tag="ps")
                nc.tensor.matmul(h4p[:], w4[:], h3T[:], start=True, stop=True)
                eT = sp.tile([D_EFF, EG], F16, tag="eT")
                nc.scalar.activation(eT[:], h4p[:], AF.Relu, bias=b4t[:])

                # aggregate: e_agg.T += e_chunk.T @ onehot_chunk
                for t in range(T):
                    ep = psp.tile([P, D_EFF], F16, tag="ps")
                    nc.tensor.transpose(
                        ep[:], eT[:, t * P : (t + 1) * P], ident16[:D_EFF, :D_EFF]
                    )
                    ec = ecp.tile([P, D_EFF], F16, tag="ec")
                    nc.scalar.copy(ec[:], ep[:])
                    first = g == 0 and t == 0
                    last = g == n_groups - 1 and t == T - 1
                    for q in range(n_obj // NQ):
                        nc.tensor.matmul(
                            agg_ps[:, q * NQ : (q + 1) * NQ],
                            ec[:],
                            oht[t][:, q * NQ : (q + 1) * NQ],
                            start=first,
                            stop=last,
                        )

            # ---- all-reduce e_agg across cores -----------------------------
            eagg_sb = const.tile([D_EFF, n_obj], F32)
            nc.scalar.copy(eagg_sb[:], agg_ps[:])
            cc_in = dp.tile([D_EFF, n_obj], F32)
            cc_out = dp.tile([D_EFF, n_obj], F32)
            nc.sync.dma_start(cc_in[:], eagg_sb[:])
            if use_collective:
                nc.gpsimd.collective_compute(
                    "AllReduce",
                    ALU.add,
                    replica_groups=[list(range(n_cores))],
                    ins=[cc_in.opt()],
                    outs=[cc_out.opt()],
                )
            else:
                nc.sync.dma_start(cc_out[:], cc_in[:])
            eaggT = const.tile([D_EFF, n_obj], F32)
            nc.sync.dma_start(eaggT[:], cc_out[:])

            # ---- node phase (object MLP) -----------------------------------
            pTt = const.tile([D_OUT, n_obj], F32)
            for q in range(n_nq):
                sl = slice(q * NQ, (q + 1) * NQ)
                cp = psp.tile([H_OBJ, NQ], F32, tag="ps")
                nc.tensor.matmul(cp[:], ow1a[:], objT[:, sl], start=True, stop=False)
                nc.tensor.matmul(cp[:], ow1b[:], eaggT[:, sl], start=False, stop=True)
                hT = sp.tile([H_OBJ, NQ], F32, tag="hT")
                nc.scalar.activation(hT[:], cp[:], AF.Relu, bias=ob1t[:])
                pp = psp.tile([D_OUT, NQ], F32, tag="ps")
                nc.tensor.matmul(pp[:], ow2[:], hT[:], start=True, stop=True)
                nc.scalar.activation(pTt[:, sl], pp[:], AF.Identity, bias=ob2t[:])
            nc.sync.dma_start(pT_d[:, :], pTt[:])

    nc.compile()
    return nc


_CACHE = {}
TRACE = False
_IOTA = np.arange(N_OBJ, dtype=np.float32)


def _get_nc():
    if "nc" not in _CACHE:
        _CACHE["nc"] = build()
    return _CACHE["nc"]


def _onehot_to_idx(a):
    """Exact index recovery from a one-hot float matrix via iota GEMV."""
    a = np.asarray(a)
    if a.dtype != np.float32:
        a = a.astype(np.float32)
    return a @ _IOTA


def _idx_blocks(v):
    """[N_REL] float indices -> [N_CORES*P, N_CHUNKS] f16, per-core chunk-major."""
    # per core: [E_PER_CORE] -> (N_CHUNKS, P) -> T -> [P, N_CHUNKS]
    return np.ascontiguousarray(
        np.transpose(v.reshape(N_CORES, N_CHUNKS, P), (0, 2, 1)).reshape(
            N_CORES * P, N_CHUNKS
        ),
        dtype=np.float16,
    )


def _get_runner():
    """Build (once) a cached jitted shard_map executable over the Bass NEFF."""
    if "runner" in _CACHE:
        return _CACHE["runner"]

    import jax
    from jax.experimental.shard_map import shard_map
    from jax.sharding import Mesh, NamedSharding, PartitionSpec

    from concourse.bass2jax import (
        _bass_exec_p,
        install_neuronx_cc_hook,
        partition_id_tensor,
    )

    nc = _get_nc()
    install_neuronx_cc_hook()
    partition_name = nc.partition_id_tensor.name if nc.partition_id_tensor else None
    in_names, out_names, out_avals, zero_outs = [], [], [], []
    for alloc in nc.m.functions[0].allocations:
        if not isinstance(alloc, mybir.MemoryLocationSet):
            continue
        name = alloc.memorylocations[0].name
        if alloc.kind == "ExternalInput":
            if name != partition_name:
                in_names.append(name)
        elif alloc.kind == "ExternalOutput":
            out_names.append(name)
            shape = tuple(alloc.tensor_shape)
            dtype = mybir.dt.np(alloc.dtype)
            out_avals.append(jax.core.ShapedArray(shape, dtype))
            zero_outs.append(np.zeros((N_CORES * shape[0], *shape[1:]), dtype))
    n_params = len(in_names)
    n_outs = len(out_avals)
    param_names = list(in_names)
    in_names = in_names + out_names
    if partition_name is not None:
        in_names.append(partition_name)
    # pT is fully written by the kernel, so the pre-zeroed output operand can
    # be uploaded once and reused (no donation)

    def _body(*args):
        operands = list(args)
        if partition_name is not None:
            operands.append(partition_id_tensor())
        outs = _bass_exec_p.bind(
            *operands,
            out_avals=tuple(out_avals),
            in_names=tuple(in_names),
            out_names=tuple(out_names),
            lowering_input_output_aliases=(),
            sim_require_finite=True,
            sim_require_nnan=True,
            nc=nc,
        )
        return tuple(outs)

    devices = jax.devices()[:N_CORES]
    mesh = Mesh(np.asarray(devices), ("core",))
    shard = NamedSharding(mesh, PartitionSpec("core"))
    repl = NamedSharding(mesh, PartitionSpec())
    param_specs = tuple(
        PartitionSpec("core") if n in _SHARDED_INPUTS else PartitionSpec()
        for n in param_names
    )
    in_specs = param_specs + (PartitionSpec("core"),) * n_outs
    out_specs = (PartitionSpec("core"),) * n_outs
    sharded = jax.jit(
        shard_map(_body, mesh=mesh, in_specs=in_specs, out_specs=out_specs,
                  check_rep=False),
        keep_unused=True,
    )
    zeros_dev = [jax.device_put(z, shard) for z in zero_outs]
    runner = dict(
        jax=jax, sharded=sharded, param_names=param_names,
        zeros_dev=zeros_dev, shard=shard, repl=repl, out_names=out_names,
    )
    _CACHE["runner"] = runner
    return runner


def kernel(**inputs):
    if not axon_active():
        return _kernel_fallback(**inputs)
    try:
        return _kernel_fast(**inputs)
    except Exception:
        _CACHE.pop("runner", None)
        return _kernel_fallback(**inputs)


def _kernel_fast(**inputs):
    r = _get_runner()
    jax = r["jax"]
    shard, repl = r["shard"], r["repl"]

    # 1) issue the index-independent uploads first (all async); the wire-heavy
    #    raT goes out before the host busies itself with the GEMVs
    devs = {}
    ra = np.asarray(inputs["ra"])
    raT = np.ascontiguousarray(
        np.transpose(ra.reshape(N_CORES, E_PER_CORE, D_REL), (0, 2, 1)).reshape(
            N_CORES * D_REL, E_PER_CORE
        ),
        dtype=np.float16,
    )
    devs["raT_c"] = jax.device_put(raT, shard)
    devs["obj"] = jax.device_put(
        np.asarray(inputs["obj"]).astype(np.float16), repl)
    devs["wpack"] = jax.device_put(_pack_weights(inputs), repl)

    # 2) index extraction overlaps with the transfers above
    recv = _onehot_to_idx(inputs["rr"])
    send = _onehot_to_idx(inputs["rs"])
    idx = np.concatenate([_idx_blocks(recv), _idx_blocks(send)], axis=1)
    devs["idx_c"] = jax.device_put(idx, shard)

    # 3) dispatch + single sync, fetching only core 0's output shard
    out_arrs = r["sharded"](*[devs[n] for n in r["param_names"]], *r["zeros_dev"])
    _CACHE["last_results"] = None
    pT0 = np.asarray(
        out_arrs[r["out_names"].index("pT")].addressable_shards[0].data
    )
    return np.ascontiguousarray(pT0.T)


def _kernel_fallback(**inputs):
    """Non-axon path: run through bass_utils with per-core input maps."""
    nc = _get_nc()
    shared = {
        "obj": np.asarray(inputs["obj"]).astype(np.float16),
        "wpack": _pack_weights(inputs),
    }
    recv = _onehot_to_idx(inputs["rr"])
    send = _onehot_to_idx(inputs["rs"])
    idx = np.concatenate([_idx_blocks(recv), _idx_blocks(send)], axis=1)
    ra = np.asarray(inputs["ra"])
    in_maps = []
    for c in range(N_CORES):
        sl = slice(c * E_PER_CORE, (c + 1) * E_PER_CORE)
        m = dict(shared)
        m["idx_c"] = np.ascontiguousarray(idx[c * P : (c + 1) * P, :])
        m["raT_c"] = np.ascontiguousarray(ra[sl].T, dtype=np.float16)
        in_maps.append(m)
    res = run_bass_kernel_spmd(
        nc, in_maps, core_ids=list(range(N_CORES)), trace=TRACE
    )
    _CACHE["last_results"] = res
    return np.ascontiguousarray(res.results[0]["pT"].T)


# revision 23
# speedup vs baseline: 69.5834x; 1.3893x over previous
"""InteractionNetwork (GNN message passing) Bass kernel for 8 Trainium2 cores.

Strategy (edge-sharded, per sharding hint):
  - The rr/rs one-hot matrices are a dense encoding of receiver/sender index
    vectors. The host losslessly re-encodes them as indices (exact GEMV
    against an iota vector), so each call ships ~3 MB instead of ~540 MB
    through the PJRT tunnel.
  - Edges are sharded across 8 cores (4096 each). On device, per 128-edge
    chunk: receiver/sender node features are gathered with indirect DMA,
    the receiver one-hot chunk [128, n_obj] is rebuilt on-chip with a
    tensor_scalar is_equal against a free-dim iota (VectorE), the 4-layer
    relation MLP runs feature-major on the PE, and edge effects are
    aggregated to nodes with e_agg.T += e_chunk.T @ onehot_chunk into a
    pinned PSUM accumulator.
  - Partial e_agg is AllReduce-summed across the 8 cores; every core then
    runs the small object MLP on all 2048 nodes; host takes core 0's output.
  - The axon tunnel is latency-bound (~70 ms/sync, ~50 MB/s), so the host
    caches the jitted executable across calls, sends the replicated
    weights/obj with a replicated sharding (1x wire bytes, not 8x), ships
    obj/ra/idx as f16, issues all transfers asynchronously (overlapped with
    the index-extraction GEMVs), and syncs exactly once on the output.
"""

import os
import sys

import numpy as np

os.environ.setdefault("MYCRO_LOCAL_CACHE", "1")
for _p in ("/opt/trn_rl_repo",):
    if os.path.isdir(_p) and _p not in sys.path:
        sys.path.insert(0, _p)

import concourse.bacc as bacc
import concourse.bass as bass
import concourse.mybir as mybir
import concourse.tile as tile
from concourse._compat import axon_active
from concourse.bass_utils import run_bass_kernel_spmd
from concourse.masks import make_identity

P = 128
F32 = mybir.dt.float32
F16 = mybir.dt.float16
I32 = mybir.dt.int32
I16 = mybir.dt.int16
AF = mybir.ActivationFunctionType
ALU = mybir.AluOpType

N_OBJ, N_REL = 2048, 32768
D_OBJ, D_REL, D_EFF = 64, 32, 64
H_REL, H_OBJ = 128, 128
D_OUT = 3
N_CORES = 8
E_PER_CORE = N_REL // N_CORES
N_CHUNKS = E_PER_CORE // P  # 32

# every input travels sharded (1x wire bytes through the latency-bound
# tunnel); wpack/obj are reassembled on device with an AllGather
_SHARDED_INPUTS = {"idx_c", "raT_c", "wpack", "obj"}

# all small weight/bias tensors travel as one packed f32 blob (one RPC)
_WPACK_LAYOUT = [
    ("rm_w1", (2 * D_OBJ + D_REL, H_REL)),
    ("rm_w2", (H_REL, H_REL)),
    ("rm_w3", (H_REL, H_REL)),
    ("rm_w4", (H_REL, D_EFF)),
    ("om_w1", (D_OBJ + D_EFF, H_OBJ)),
    ("om_w2", (H_OBJ, D_OUT)),
    ("rm_b1", (H_REL,)),
    ("rm_b2", (H_REL,)),
    ("rm_b3", (H_REL,)),
    ("rm_b4", (D_EFF,)),
    ("om_b1", (H_OBJ,)),
    ("om_b2", (D_OUT,)),
]
_WPACK_OFF = {}
_o = 0
for _n, _s in _WPACK_LAYOUT:
    _WPACK_OFF[_n] = _o
    _o += int(np.prod(_s))
_WPACK_SIZE = ((_o + N_CORES - 1) // N_CORES) * N_CORES  # pad to 8 shards


def _pack_weights(inputs):
    out = np.zeros(_WPACK_SIZE, np.float32)
    for n, s in _WPACK_LAYOUT:
        a = np.asarray(inputs[n])
        if a.dtype != np.float32:
            a = a.astype(np.float32)
        out[_WPACK_OFF[n] : _WPACK_OFF[n] + a.size] = a.ravel()
    return out


def build(n_cores=N_CORES, e_per_core=E_PER_CORE, n_obj=N_OBJ,
          use_collective=True):
    EG = 512                  # edges per MLP group
    T = EG // P               # 128-edge chunks per group
    n_groups = e_per_core // EG
    n_chunks = e_per_core // P
    NQ = 512                  # node chunk (psum bank) for wide matmuls
    n_nq = n_obj // NQ

    nc = bacc.Bacc(
        "TRN2",
        target_bir_lowering=False,
        debug=False,
        enable_asserts=False,
        num_devices=n_cores,
    )

    idx = nc.dram_tensor("idx_c", [P, 2 * n_chunks], F16, kind="ExternalInput")
    raT = nc.dram_tensor("raT_c", [D_REL, e_per_core], F16, kind="ExternalInput")
    obj_s = nc.dram_tensor("obj", [n_obj * D_OBJ // n_cores], F16, kind="ExternalInput")
    wpack_s = nc.dram_tensor("wpack", [_WPACK_SIZE // n_cores], F32, kind="ExternalInput")
    pT_d = nc.dram_tensor("pT", [D_OUT, n_obj], F32, kind="ExternalOutput")

    with tile.TileContext(nc) as tc:
        with (
            tc.tile_pool(name="const", bufs=1) as const,
            tc.tile_pool(name="stream", bufs=8) as sp,
            tc.tile_pool(name="gat", bufs=4) as gp,
            tc.tile_pool(name="ec", bufs=8) as ecp,
            tc.tile_pool(name="aggp", bufs=1, space="PSUM") as aggp,
            tc.tile_pool(name="psp", bufs=4, space="PSUM") as psp,
            tc.tile_pool(name="dram", bufs=1, space="DRAM") as dp,
        ):
            # ---- reassemble the sharded weight/obj packs (1x wire bytes) ---
            # collectives cannot read IO tensors; stage shards into internal
            # DRAM first
            wstage = dp.tile([_WPACK_SIZE // n_cores], F32)
            nc.sync.dma_start(wstage[:], wpack_s[:])
            ostage = dp.tile([n_obj * D_OBJ // n_cores], F16)
            nc.sync.dma_start(ostage[:], obj_s[:])
            wfull = dp.tile([_WPACK_SIZE], F32)
            nc.gpsimd.collective_compute(
                "AllGather",
                ALU.bypass,
                replica_groups=[list(range(n_cores))],
                ins=[wstage[:]],
                outs=[wfull[:]],
            )
            ofull = dp.tile([n_obj * D_OBJ], F16)
            nc.gpsimd.collective_compute(
                "AllGather",
                ALU.bypass,
                replica_groups=[list(range(n_cores))],
                ins=[ostage[:]],
                outs=[ofull[:]],
            )
            obj = ofull[:].rearrange("(n d) -> n d", n=n_obj, d=D_OBJ)

            def wview(name, r0, r1):
                """2-D AP over the gathered pack: rows [r0:r1) of `name`."""
                shape = dict(_WPACK_LAYOUT)[name]
                cols = shape[1] if len(shape) == 2 else 1
                o = _WPACK_OFF[name] + r0 * cols
                return wfull[o : o + (r1 - r0) * cols].rearrange(
                    "(a b) -> a b", a=r1 - r0, b=cols
                )

            # ---- constants -------------------------------------------------
            ident32 = const.tile([P, P], F32)
            make_identity(nc, ident32[:])
            ident16 = const.tile([P, P], F16)
            make_identity(nc, ident16[:])

            iota_i = const.tile([P, n_obj], I16)
            nc.gpsimd.iota(iota_i[:], pattern=[[1, n_obj]], base=0, channel_multiplier=0)
            iota16 = const.tile([P, n_obj], F16)
            nc.vector.tensor_copy(iota16[:], iota_i[:])

            idx_sb16 = const.tile([P, 2 * n_chunks], F16)
            nc.sync.dma_start(idx_sb16[:], idx[:, :])
            idx_sb = const.tile([P, 2 * n_chunks], I32)
            nc.vector.tensor_copy(idx_sb[:], idx_sb16[:])
            idxf32 = const.tile([P, n_chunks], F32)
            nc.vector.tensor_copy(idxf32[:], idx_sb16[:, 0:n_chunks])

            w1ab = const.tile([P, H_REL], F32)
            nc.sync.dma_start(w1ab[:], wview("rm_w1", 0, P))
            w1c = const.tile([D_REL, H_REL], F32)
            nc.sync.dma_start(w1c[:], wview("rm_w1", P, P + D_REL))
            w2 = const.tile([H_REL, H_REL], F32)
            nc.sync.dma_start(w2[:], wview("rm_w2", 0, H_REL))
            w3 = const.tile([H_REL, H_REL], F32)
            nc.sync.dma_start(w3[:], wview("rm_w3", 0, H_REL))
            w4 = const.tile([H_REL, D_EFF], F32)
            nc.sync.dma_start(w4[:], wview("rm_w4", 0, H_REL))
            b1t = const.tile([H_REL, 1], F32)
            nc.sync.dma_start(b1t[:], wview("rm_b1", 0, H_REL))
            b2t = const.tile([H_REL, 1], F32)
            nc.sync.dma_start(b2t[:], wview("rm_b2", 0, H_REL))
            b3t = const.tile([H_REL, 1], F32)
            nc.sync.dma_start(b3t[:], wview("rm_b3", 0, H_REL))
            b4t = const.tile([D_EFF, 1], F32)
            nc.sync.dma_start(b4t[:], wview("rm_b4", 0, D_EFF))
            ow1a = const.tile([D_OBJ, H_OBJ], F32)
            nc.sync.dma_start(ow1a[:], wview("om_w1", 0, D_OBJ))
            ow1b = const.tile([D_EFF, H_OBJ], F32)
            nc.sync.dma_start(ow1b[:], wview("om_w1", D_OBJ, D_OBJ + D_EFF))
            ow2 = const.tile([H_OBJ, D_OUT], F32)
            nc.sync.dma_start(ow2[:], wview("om_w2", 0, H_OBJ))
            ob1t = const.tile([H_OBJ, 1], F32)
            nc.sync.dma_start(ob1t[:], wview("om_b1", 0, H_OBJ))
            ob2t = const.tile([D_OUT, 1], F32)
            nc.sync.dma_start(ob2t[:], wview("om_b2", 0, D_OUT))

            # obj.T in SBUF (for the node-model MLP), f16 -> f32
            objT = const.tile([D_OBJ, n_obj], F32)
            for k in range(n_obj // P):
                ot = gp.tile([P, D_OBJ], F16, tag="objload")
                nc.sync.dma_start(
                    ot[:],
                    ofull[k * P * D_OBJ : (k + 1) * P * D_OBJ].rearrange(
                        "(a b) -> a b", a=P, b=D_OBJ
                    ),
                )
                tp = psp.tile([D_OBJ, P], F16, tag="ps")
                nc.tensor.transpose(tp[:], ot[:], ident16[:])
                nc.scalar.copy(objT[:, k * P : (k + 1) * P], tp[:])

            # pinned accumulator: e_agg.T [64, n_obj] (4 PSUM banks)
            agg_ps = aggp.tile([D_EFF, n_obj], F32)

            # ---- edge phase ------------------------------------------------
            for g in range(n_groups):
                e0 = g * EG
                oht = []
                for t in range(T):
                    c = g * T + t
                    oh = sp.tile([P, n_obj], F16, tag="oh")
                    nc.vector.tensor_scalar(
                        oh[:], iota16[:], idxf32[:, c : c + 1], None,
                        op0=ALU.is_equal,
                    )
                    oht.append(oh)

                raTg16 = sp.tile([D_REL, EG], F16, tag="raT16")
                nc.sync.dma_start(raTg16[:], raT[:, e0 : e0 + EG])
                raTg = sp.tile([D_REL, EG], F32, tag="raT")
                nc.vector.tensor_copy(raTg[:], raTg16[:])

                b1T = sp.tile([P, EG], F32, tag="b1T")
                for t in range(T):
                    c = g * T + t
                    orr_t = gp.tile([P, D_OBJ], F16, tag="gat")
                    nc.gpsimd.indirect_dma_start(
                        out=orr_t[:], out_offset=None, in_=obj,
                        in_offset=bass.IndirectOffsetOnAxis(
                            ap=idx_sb[:, c : c + 1], axis=0
                        ),
                    )
                    tp = psp.tile([D_OBJ, P], F16, tag="ps")
                    nc.tensor.transpose(tp[:], orr_t[:], ident16[:])
                    nc.scalar.copy(b1T[0:D_OBJ, t * P : (t + 1) * P], tp[:])

                    ors_t = gp.tile([P, D_OBJ], F16, tag="gat")
                    nc.gpsimd.indirect_dma_start(
                        out=ors_t[:], out_offset=None, in_=obj,
                        in_offset=bass.IndirectOffsetOnAxis(
                            ap=idx_sb[:, n_chunks + c : n_chunks + c + 1], axis=0
                        ),
                    )
                    tp2 = psp.tile([D_OBJ, P], F16, tag="ps")
                    nc.tensor.transpose(tp2[:], ors_t[:], ident16[:])
                    nc.scalar.copy(b1T[D_OBJ : 2 * D_OBJ, t * P : (t + 1) * P], tp2[:])

                # relation MLP, feature-major [features, EG]
                h1p = psp.tile([H_REL, EG], F32, tag="ps")
                nc.tensor.matmul(h1p[:], w1ab[:], b1T[:], start=True, stop=False)
                nc.tensor.matmul(h1p[:], w1c[:], raTg[:], start=False, stop=True)
                h1T = sp.tile([H_REL, EG], F32, tag="hT")
                nc.scalar.activation(h1T[:], h1p[:], AF.Relu, bias=b1t[:])

                h2p = psp.tile([H_REL, EG], F32, tag="ps")
                nc.tensor.matmul(h2p[:], w2[:], h1T[:], start=True, stop=True)
                h2T = sp.tile([H_REL, EG], F32, tag="hT")
                nc.scalar.activation(h2T[:], h2p[:], AF.Relu, bias=b2t[:])

                h3p = psp.tile([H_REL, EG], F32, tag="ps")
                nc.tensor.matmul(h3p[:], w3[:], h2T[:], start=True, stop=True)
                h3T = sp.tile([H_REL, EG], F32, tag="hT")
                nc.scalar.activation(h3T[:], h3p[:], AF.Relu, bias=b3t[:])

                h4p = psp.tile([D_EFF, EG], F32, tag="ps")
                nc.tensor.matmul(h4p[:], w4[:], h3T[:], start=True, stop=True)
                eT = sp.tile([D_EFF, EG], F16, tag="eT")
                nc.scalar.activation(eT[:], h4p[:], AF.Relu, bias=b4t[:])

                # aggregate: e_agg.T += e_chunk.T @ onehot_chunk
                for t in range(T):
                    ep = psp.tile([P, D_EFF], F16, tag="ps")
                    nc.tensor.transpose(
                        ep[:], eT[:, t * P : (t + 1) * P], ident16[:D_EFF, :D_EFF]
                    )
                    ec = ecp.tile([P, D_EFF], F16, tag="ec")
                    nc.scalar.copy(ec[:], ep[:])
                    first = g == 0 and t == 0
                    last = g == n_groups - 1 and t == T - 1
                    for q in range(n_obj // NQ):
                        nc.tensor.matmul(
                            agg_ps[:, q * NQ : (q + 1) * NQ],
                            ec[:],
                            oht[t][:, q * NQ : (q + 1) * NQ],
                            start=first,
                            stop=last,
                        )

            # ---- all-reduce e_agg across cores -----------------------------
            eagg_sb = const.tile([D_EFF, n_obj], F32)
            nc.scalar.copy(eagg_sb[:], agg_ps[:])
            cc_in = dp.tile([D_EFF, n_obj], F32)
            cc_out = dp.tile([D_EFF, n_obj], F32)
            nc.sync.dma_start(cc_in[:], eagg_sb[:])
            if use_collective:
                nc.gpsimd.collective_compute(
                    "AllReduce",
                    ALU.add,
                    replica_groups=[list(range(n_cores))],
                    ins=[cc_in.opt()],
                    outs=[cc_out.opt()],
                )
            else:
                nc.sync.dma_start(cc_out[:], cc_in[:])
            eaggT = const.tile([D_EFF, n_obj], F32)
            nc.sync.dma_start(eaggT[:], cc_out[:])

            # ---- node phase (object MLP) -----------------------------------
            pTt = const.tile([D_OUT, n_obj], F32)
            for q in range(n_nq):
                sl = slice(q * NQ, (q + 1) * NQ)
                cp = psp.tile([H_OBJ, NQ], F32, tag="ps")
                nc.tensor.matmul(cp[:], ow1a[:], objT[:, sl], start=True, stop=False)
                nc.tensor.matmul(cp[:], ow1b[:], eaggT[:, sl], start=False, stop=True)
                hT = sp.tile([H_OBJ, NQ], F32, tag="hT")
                nc.scalar.activation(hT[:], cp[:], AF.Relu, bias=ob1t[:])
                pp = psp.tile([D_OUT, NQ], F32, tag="ps")
                nc.tensor.matmul(pp[:], ow2[:], hT[:], start=True, stop=True)
                nc.scalar.activation(pTt[:, sl], pp[:], AF.Identity, bias=ob2t[:])
            nc.sync.dma_start(pT_d[:, :], pTt[:])

    nc.compile()
    return nc


_CACHE = {}
TRACE = False
_IOTA = np.arange(N_OBJ, dtype=np.float32)


def _get_nc():
    if "nc" not in _CACHE:
        _CACHE["nc"] = build()
    return _CACHE["nc"]


def _onehot_to_idx(a):
    """Exact index recovery from a one-hot float matrix via iota GEMV."""
    a = np.asarray(a)
    if a.dtype != np.float32:
        a = a.astype(np.float32)
    return a @ _IOTA


def _idx_blocks(v):
    """[N_REL] float indices -> [N_CORES*P, N_CHUNKS] f16, per-core chunk-major."""
    # per core: [E_PER_CORE] -> (N_CHUNKS, P) -> T -> [P, N_CHUNKS]
    return np.ascontiguousarray(
        np.transpose(v.reshape(N_CORES, N_CHUNKS, P), (0, 2, 1)).reshape(
            N_CORES * P, N_CHUNKS
        ),
        dtype=np.float16,
    )


def _get_runner():
    """Build (once) a cached jitted shard_map executable over the Bass NEFF."""
    if "runner" in _CACHE:
        return _CACHE["runner"]

    import jax
    from jax.experimental.shard_map import shard_map
    from jax.sharding import Mesh, NamedSharding, PartitionSpec

    from concourse.bass2jax import (
        _bass_exec_p,
        install_neuronx_cc_hook,
        partition_id_tensor,
    )

    nc = _get_nc()
    install_neuronx_cc_hook()
    partition_name = nc.partition_id_tensor.name if nc.partition_id_tensor else None
    in_names, out_names, out_avals, zero_outs = [], [], [], []
    for alloc in nc.m.functions[0].allocations:
        if not isinstance(alloc, mybir.MemoryLocationSet):
            continue
        name = alloc.memorylocations[0].name
        if alloc.kind == "ExternalInput":
            if name != partition_name:
                in_names.append(name)
        elif alloc.kind == "ExternalOutput":
            out_names.append(name)
            shape = tuple(alloc.tensor_shape)
            dtype = mybir.dt.np(alloc.dtype)
            out_avals.append(jax.core.ShapedArray(shape, dtype))
            zero_outs.append(np.zeros((N_CORES * shape[0], *shape[1:]), dtype))
    n_params = len(in_names)
    n_outs = len(out_avals)
    param_names = list(in_names)
    in_names = in_names + out_names
    if partition_name is not None:
        in_names.append(partition_name)
    # pT is fully written by the kernel, so the pre-zeroed output operand can
    # be uploaded once and reused (no donation)

    def _body(*args):
        operands = list(args)
        if partition_name is not None:
            operands.append(partition_id_tensor())
        outs = _bass_exec_p.bind(
            *operands,
            out_avals=tuple(out_avals),
            in_names=tuple(in_names),
            out_names=tuple(out_names),
            lowering_input_output_aliases=(),
            sim_require_finite=True,
            sim_require_nnan=True,
            nc=nc,
        )
        return tuple(outs)

    devices = jax.devices()[:N_CORES]
    mesh = Mesh(np.asarray(devices), ("core",))
    shard = NamedSharding(mesh, PartitionSpec("core"))
    repl = NamedSharding(mesh, PartitionSpec())
    param_specs = tuple(
        PartitionSpec("core") if n in _SHARDED_INPUTS else PartitionSpec()
        for n in param_names
    )
    in_specs = param_specs + (PartitionSpec("core"),) * n_outs
    out_specs = (PartitionSpec("core"),) * n_outs
    sharded = jax.jit(
        shard_map(_body, mesh=mesh, in_specs=in_specs, out_specs=out_specs,
                  check_rep=False),
        keep_unused=True,
    )
    zeros_dev = [jax.device_put(z, shard) for z in zero_outs]
    runner = dict(
        jax=jax, sharded=sharded, param_names=param_names,
        zeros_dev=zeros_dev, shard=shard, repl=repl, out_names=out_names,
    )
    _CACHE["runner"] = runner
    return runner


def kernel(**inputs):
    if not axon_active():
        return _kernel_fallback(**inputs)
    try:
        return _kernel_fast(**inputs)
    except Exception:
        _CACHE.pop("runner", None)
        return _kernel_fallback(**inputs)


def _kernel_fast(**inputs):
    r = _get_runner()
    jax = r["jax"]
    shard, repl = r["shard"], r["repl"]

    # 1) issue the index-independent uploads first (all async); the wire-heavy
    #    raT goes out before the host busies itself with the GEMVs
    devs = {}
    ra = np.asarray(inputs["ra"])
    raT = np.ascontiguousarray(
        np.transpose(ra.reshape(N_CORES, E_PER_CORE, D_REL), (0, 2, 1)).reshape(
            N_CORES * D_REL, E_PER_CORE
        ),
        dtype=np.float16,
    )
    devs["raT_c"] = jax.device_put(raT, shard)
    devs["obj"] = jax.device_put(
        np.asarray(inputs["obj"]).astype(np.float16).ravel(), shard)
    devs["wpack"] = jax.device_put(_pack_weights(inputs), shard)

    # 2) index extraction overlaps with the transfers above
    recv = _onehot_to_idx(inputs["rr"])
    send = _onehot_to_idx(inputs["rs"])
    idx = np.concatenate([_idx_blocks(recv), _idx_blocks(send)], axis=1)
    devs["idx_c"] = jax.device_put(idx, shard)

    # 3) dispatch + single sync, fetching only core 0's output shard
    out_arrs = r["sharded"](*[devs[n] for n in r["param_names"]], *r["zeros_dev"])
    _CACHE["last_results"] = None
    pT0 = np.asarray(
        out_arrs[r["out_names"].index("pT")].addressable_shards[0].data
    )
    return np.ascontiguousarray(pT0.T)


def _kernel_fallback(**inputs):
    """Non-axon path: run through bass_utils with per-core input maps."""
    nc = _get_nc()
    objflat = np.asarray(inputs["obj"]).astype(np.float16).ravel()
    wpack = _pack_weights(inputs)
    recv = _onehot_to_idx(inputs["rr"])
    send = _onehot_to_idx(inputs["rs"])
    idx = np.concatenate([_idx_blocks(recv), _idx_blocks(send)], axis=1)
    ra = np.asarray(inputs["ra"])
    osh = objflat.size // N_CORES
    wsh = wpack.size // N_CORES
    in_maps = []
    for c in range(N_CORES):
        sl = slice(c * E_PER_CORE, (c + 1) * E_PER_CORE)
        m = {
            "obj": objflat[c * osh : (c + 1) * osh],
            "wpack": wpack[c * wsh : (c + 1) * wsh],
            "idx_c": np.ascontiguousarray(idx[c * P : (c + 1) * P, :]),
            "raT_c": np.ascontiguousarray(ra[sl].T, dtype=np.float16),
        }
        in_maps.append(m)
    res = run_bass_kernel_spmd(
        nc, in_maps, core_ids=list(range(N_CORES)), trace=TRACE
    )
    _CACHE["last_results"] = res
    return np.ascontiguousarray(res.results[0]["pT"].T)


# revision 25
# speedup vs baseline: 70.6121x; 1.0148x over previous
"""InteractionNetwork (GNN message passing) Bass kernel for 8 Trainium2 cores.

Strategy (edge-sharded, per sharding hint):
  - The rr/rs one-hot matrices are a dense encoding of receiver/sender index
    vectors. The host losslessly re-encodes them as indices (exact GEMV
    against an iota vector), so each call ships ~3 MB instead of ~540 MB
    through the PJRT tunnel.
  - Edges are sharded across 8 cores (4096 each). On device, per 128-edge
    chunk: receiver/sender node features are gathered with indirect DMA,
    the receiver one-hot chunk [128, n_obj] is rebuilt on-chip with a
    tensor_scalar is_equal against a free-dim iota (VectorE), the 4-layer
    relation MLP runs feature-major on the PE, and edge effects are
    aggregated to nodes with e_agg.T += e_chunk.T @ onehot_chunk into a
    pinned PSUM accumulator.
  - Partial e_agg is AllReduce-summed across the 8 cores; every core then
    runs the small object MLP on all 2048 nodes; host takes core 0's output.
  - The axon tunnel is latency-bound (~70 ms/sync, ~50 MB/s), so the host
    caches the jitted executable across calls, sends the replicated
    weights/obj with a replicated sharding (1x wire bytes, not 8x), ships
    obj/ra/idx as f16, issues all transfers asynchronously (overlapped with
    the index-extraction GEMVs), and syncs exactly once on the output.
"""

import os
import sys

import numpy as np

os.environ.setdefault("MYCRO_LOCAL_CACHE", "1")
for _p in ("/opt/trn_rl_repo",):
    if os.path.isdir(_p) and _p not in sys.path:
        sys.path.insert(0, _p)

import concourse.bacc as bacc
import concourse.bass as bass
import concourse.mybir as mybir
import concourse.tile as tile
from concourse._compat import axon_active
from concourse.bass_utils import run_bass_kernel_spmd
from concourse.masks import make_identity

P = 128
F32 = mybir.dt.float32
F16 = mybir.dt.float16
I32 = mybir.dt.int32
I16 = mybir.dt.int16
AF = mybir.ActivationFunctionType
ALU = mybir.AluOpType

N_OBJ, N_REL = 2048, 32768
D_OBJ, D_REL, D_EFF = 64, 32, 64
H_REL, H_OBJ = 128, 128
D_OUT = 3
N_CORES = 8
E_PER_CORE = N_REL // N_CORES
N_CHUNKS = E_PER_CORE // P  # 32

# every input travels sharded (1x wire bytes through the latency-bound
# tunnel); wpack/obj are reassembled on device with an AllGather
_SHARDED_INPUTS = {"idx_c", "raT_c", "wpack", "obj"}

# all small weight/bias tensors travel as one packed f32 blob (one RPC)
_WPACK_LAYOUT = [
    ("rm_w1", (2 * D_OBJ + D_REL, H_REL)),
    ("rm_w2", (H_REL, H_REL)),
    ("rm_w3", (H_REL, H_REL)),
    ("rm_w4", (H_REL, D_EFF)),
    ("om_w1", (D_OBJ + D_EFF, H_OBJ)),
    ("om_w2", (H_OBJ, D_OUT)),
    ("rm_b1", (H_REL,)),
    ("rm_b2", (H_REL,)),
    ("rm_b3", (H_REL,)),
    ("rm_b4", (D_EFF,)),
    ("om_b1", (H_OBJ,)),
    ("om_b2", (D_OUT,)),
]
_WPACK_OFF = {}
_o = 0
for _n, _s in _WPACK_LAYOUT:
    _WPACK_OFF[_n] = _o
    _o += int(np.prod(_s))
_WPACK_SIZE = ((_o + N_CORES - 1) // N_CORES) * N_CORES  # pad to 8 shards


def _pack_weights(inputs):
    out = np.zeros(_WPACK_SIZE, np.float32)
    for n, s in _WPACK_LAYOUT:
        a = np.asarray(inputs[n])
        if a.dtype != np.float32:
            a = a.astype(np.float32)
        out[_WPACK_OFF[n] : _WPACK_OFF[n] + a.size] = a.ravel()
    return out


def build(n_cores=N_CORES, e_per_core=E_PER_CORE, n_obj=N_OBJ,
          use_collective=True):
    EG = 512                  # edges per MLP group
    T = EG // P               # 128-edge chunks per group
    n_groups = e_per_core // EG
    n_chunks = e_per_core // P
    NQ = 512                  # node chunk (psum bank) for wide matmuls
    n_nq = n_obj // NQ

    nc = bacc.Bacc(
        "TRN2",
        target_bir_lowering=False,
        debug=False,
        enable_asserts=False,
        num_devices=n_cores,
    )

    idx = nc.dram_tensor("idx_c", [P, 2 * n_chunks], F16, kind="ExternalInput")
    raT = nc.dram_tensor("raT_c", [D_REL, e_per_core], F16, kind="ExternalInput")
    obj_s = nc.dram_tensor("obj", [n_obj * D_OBJ // n_cores], F16, kind="ExternalInput")
    wpack_s = nc.dram_tensor("wpack", [_WPACK_SIZE // n_cores], F32, kind="ExternalInput")
    pT_d = nc.dram_tensor("pT", [D_OUT, n_obj], F32, kind="ExternalOutput")

    with tile.TileContext(nc) as tc:
        with (
            tc.tile_pool(name="const", bufs=1) as const,
            tc.tile_pool(name="stream", bufs=8) as sp,
            tc.tile_pool(name="gat", bufs=4) as gp,
            tc.tile_pool(name="ec", bufs=8) as ecp,
            tc.tile_pool(name="aggp", bufs=1, space="PSUM") as aggp,
            tc.tile_pool(name="psp", bufs=4, space="PSUM") as psp,
            tc.tile_pool(name="dram", bufs=1, space="DRAM") as dp,
        ):
            # ---- reassemble the sharded weight/obj packs (1x wire bytes) ---
            # collectives cannot read IO tensors; stage shards into internal
            # DRAM first
            wstage = dp.tile([_WPACK_SIZE // n_cores], F32)
            nc.sync.dma_start(wstage[:], wpack_s[:])
            ostage = dp.tile([n_obj * D_OBJ // n_cores], F16)
            nc.sync.dma_start(ostage[:], obj_s[:])
            wfull = dp.tile([_WPACK_SIZE], F32)
            nc.gpsimd.collective_compute(
                "AllGather",
                ALU.bypass,
                replica_groups=[list(range(n_cores))],
                ins=[wstage[:]],
                outs=[wfull[:]],
            )
            ofull = dp.tile([n_obj * D_OBJ], F16)
            nc.gpsimd.collective_compute(
                "AllGather",
                ALU.bypass,
                replica_groups=[list(range(n_cores))],
                ins=[ostage[:]],
                outs=[ofull[:]],
            )
            obj = ofull[:].rearrange("(n d) -> n d", n=n_obj, d=D_OBJ)

            def wview(name, r0, r1):
                """2-D AP over the gathered pack: rows [r0:r1) of `name`."""
                shape = dict(_WPACK_LAYOUT)[name]
                cols = shape[1] if len(shape) == 2 else 1
                o = _WPACK_OFF[name] + r0 * cols
                return wfull[o : o + (r1 - r0) * cols].rearrange(
                    "(a b) -> a b", a=r1 - r0, b=cols
                )

            # ---- constants -------------------------------------------------
            ident32 = const.tile([P, P], F32)
            make_identity(nc, ident32[:])
            ident16 = const.tile([P, P], F16)
            make_identity(nc, ident16[:])

            iota_i = const.tile([P, n_obj], I16)
            nc.gpsimd.iota(iota_i[:], pattern=[[1, n_obj]], base=0, channel_multiplier=0)
            iota16 = const.tile([P, n_obj], F16)
            nc.vector.tensor_copy(iota16[:], iota_i[:])

            idx_sb16 = const.tile([P, 2 * n_chunks], F16)
            nc.sync.dma_start(idx_sb16[:], idx[:, :])
            idx_sb = const.tile([P, 2 * n_chunks], I32)
            nc.vector.tensor_copy(idx_sb[:], idx_sb16[:])
            idxf32 = const.tile([P, n_chunks], F32)
            nc.vector.tensor_copy(idxf32[:], idx_sb16[:, 0:n_chunks])

            w1ab = const.tile([P, H_REL], F32)
            nc.sync.dma_start(w1ab[:], wview("rm_w1", 0, P))
            w1c = const.tile([D_REL, H_REL], F32)
            nc.sync.dma_start(w1c[:], wview("rm_w1", P, P + D_REL))
            w2 = const.tile([H_REL, H_REL], F32)
            nc.sync.dma_start(w2[:], wview("rm_w2", 0, H_REL))
            w3 = const.tile([H_REL, H_REL], F32)
            nc.sync.dma_start(w3[:], wview("rm_w3", 0, H_REL))
            w4 = const.tile([H_REL, D_EFF], F32)
            nc.sync.dma_start(w4[:], wview("rm_w4", 0, H_REL))
            b1t = const.tile([H_REL, 1], F32)
            nc.sync.dma_start(b1t[:], wview("rm_b1", 0, H_REL))
            b2t = const.tile([H_REL, 1], F32)
            nc.sync.dma_start(b2t[:], wview("rm_b2", 0, H_REL))
            b3t = const.tile([H_REL, 1], F32)
            nc.sync.dma_start(b3t[:], wview("rm_b3", 0, H_REL))
            b4t = const.tile([D_EFF, 1], F32)
            nc.sync.dma_start(b4t[:], wview("rm_b4", 0, D_EFF))
            ow1a = const.tile([D_OBJ, H_OBJ], F32)
            nc.sync.dma_start(ow1a[:], wview("om_w1", 0, D_OBJ))
            ow1b = const.tile([D_EFF, H_OBJ], F32)
            nc.sync.dma_start(ow1b[:], wview("om_w1", D_OBJ, D_OBJ + D_EFF))
            ow2 = const.tile([H_OBJ, D_OUT], F32)
            nc.sync.dma_start(ow2[:], wview("om_w2", 0, H_OBJ))
            ob1t = const.tile([H_OBJ, 1], F32)
            nc.sync.dma_start(ob1t[:], wview("om_b1", 0, H_OBJ))
            ob2t = const.tile([D_OUT, 1], F32)
            nc.sync.dma_start(ob2t[:], wview("om_b2", 0, D_OUT))

            # obj.T in SBUF (for the node-model MLP), f16 -> f32
            objT = const.tile([D_OBJ, n_obj], F32)
            for k in range(n_obj // P):
                ot = gp.tile([P, D_OBJ], F16, tag="objload")
                nc.sync.dma_start(
                    ot[:],
                    ofull[k * P * D_OBJ : (k + 1) * P * D_OBJ].rearrange(
                        "(a b) -> a b", a=P, b=D_OBJ
                    ),
                )
                tp = psp.tile([D_OBJ, P], F16, tag="ps")
                nc.tensor.transpose(tp[:], ot[:], ident16[:])
                nc.scalar.copy(objT[:, k * P : (k + 1) * P], tp[:])

            # pinned accumulator: e_agg.T [64, n_obj] (4 PSUM banks)
            agg_ps = aggp.tile([D_EFF, n_obj], F32)

            # ---- edge phase ------------------------------------------------
            for g in range(n_groups):
                e0 = g * EG
                oht = []
                for t in range(T):
                    c = g * T + t
                    oh = sp.tile([P, n_obj], F16, tag="oh")
                    nc.vector.tensor_scalar(
                        oh[:], iota16[:], idxf32[:, c : c + 1], None,
                        op0=ALU.is_equal,
                    )
                    oht.append(oh)

                raTg16 = sp.tile([D_REL, EG], F16, tag="raT16")
                nc.sync.dma_start(raTg16[:], raT[:, e0 : e0 + EG])
                raTg = sp.tile([D_REL, EG], F32, tag="raT")
                nc.vector.tensor_copy(raTg[:], raTg16[:])

                b1T = sp.tile([P, EG], F32, tag="b1T")
                for t in range(T):
                    c = g * T + t
                    orr_t = gp.tile([P, D_OBJ], F16, tag="gat")
                    nc.gpsimd.indirect_dma_start(
                        out=orr_t[:], out_offset=None, in_=obj,
                        in_offset=bass.IndirectOffsetOnAxis(
                            ap=idx_sb[:, c : c + 1], axis=0
                        ),
                    )
                    tp = psp.tile([D_OBJ, P], F16, tag="ps")
                    nc.tensor.transpose(tp[:], orr_t[:], ident16[:])
                    nc.scalar.copy(b1T[0:D_OBJ, t * P : (t + 1) * P], tp[:])

                    ors_t = gp.tile([P, D_OBJ], F16, tag="gat")
                    nc.gpsimd.indirect_dma_start(
                        out=ors_t[:], out_offset=None, in_=obj,
                        in_offset=bass.IndirectOffsetOnAxis(
                            ap=idx_sb[:, n_chunks + c : n_chunks + c + 1], axis=0
                        ),
                    )
                    tp2 = psp.tile([D_OBJ, P], F16, tag="ps")
                    nc.tensor.transpose(tp2[:], ors_t[:], ident16[:])
                    nc.scalar.copy(b1T[D_OBJ : 2 * D_OBJ, t * P : (t + 1) * P], tp2[:])

                # relation MLP, feature-major [features, EG]
                h1p = psp.tile([H_REL, EG], F32, tag="ps")
                nc.tensor.matmul(h1p[:], w1ab[:], b1T[:], start=True, stop=False)
                nc.tensor.matmul(h1p[:], w1c[:], raTg[:], start=False, stop=True)
                h1T = sp.tile([H_REL, EG], F32, tag="hT")
                nc.scalar.activation(h1T[:], h1p[:], AF.Relu, bias=b1t[:])

                h2p = psp.tile([H_REL, EG], F32, tag="ps")
                nc.tensor.matmul(h2p[:], w2[:], h1T[:], start=True, stop=True)
                h2T = sp.tile([H_REL, EG], F32, tag="hT")
                nc.scalar.activation(h2T[:], h2p[:], AF.Relu, bias=b2t[:])

                h3p = psp.tile([H_REL, EG], F32, tag="ps")
                nc.tensor.matmul(h3p[:], w3[:], h2T[:], start=True, stop=True)
                h3T = sp.tile([H_REL, EG], F32, tag="hT")
                nc.scalar.activation(h3T[:], h3p[:], AF.Relu, bias=b3t[:])

                h4p = psp.tile([D_EFF, EG], F32, tag="ps")
                nc.tensor.matmul(h4p[:], w4[:], h3T[:], start=True, stop=True)
                eT = sp.tile([D_EFF, EG], F16, tag="eT")
                nc.scalar.activation(eT[:], h4p[:], AF.Relu, bias=b4t[:])

                # aggregate: e_agg.T += e_chunk.T @ onehot_chunk
                for t in range(T):
                    ep = psp.tile([P, D_EFF], F16, tag="ps")
                    nc.tensor.transpose(
                        ep[:], eT[:, t * P : (t + 1) * P], ident16[:D_EFF, :D_EFF]
                    )
                    ec = ecp.tile([P, D_EFF], F16, tag="ec")
                    nc.scalar.copy(ec[:], ep[:])
                    first = g == 0 and t == 0
                    last = g == n_groups - 1 and t == T - 1
                    for q in range(n_obj // NQ):
                        nc.tensor.matmul(
                            agg_ps[:, q * NQ : (q + 1) * NQ],
                            ec[:],
                            oht[t][:, q * NQ : (q + 1) * NQ],
                            start=first,
                            stop=last,
                        )

            # ---- all-reduce e_agg across cores -----------------------------
            eagg_sb = const.tile([D_EFF, n_obj], F32)
            nc.scalar.copy(eagg_sb[:], agg_ps[:])
            cc_in = dp.tile([D_EFF, n_obj], F32)
            cc_out = dp.tile([D_EFF, n_obj], F32)
            nc.sync.dma_start(cc_in[:], eagg_sb[:])
            if use_collective:
                nc.gpsimd.collective_compute(
                    "AllReduce",
                    ALU.add,
                    replica_groups=[list(range(n_cores))],
                    ins=[cc_in.opt()],
                    outs=[cc_out.opt()],
                )
            else:
                nc.sync.dma_start(cc_out[:], cc_in[:])
            eaggT = const.tile([D_EFF, n_obj], F32)
            nc.sync.dma_start(eaggT[:], cc_out[:])

            # ---- node phase (object MLP) -----------------------------------
            pTt = const.tile([D_OUT, n_obj], F32)
            for q in range(n_nq):
                sl = slice(q * NQ, (q + 1) * NQ)
                cp = psp.tile([H_OBJ, NQ], F32, tag="ps")
                nc.tensor.matmul(cp[:], ow1a[:], objT[:, sl], start=True, stop=False)
                nc.tensor.matmul(cp[:], ow1b[:], eaggT[:, sl], start=False, stop=True)
                hT = sp.tile([H_OBJ, NQ], F32, tag="hT")
                nc.scalar.activation(hT[:], cp[:], AF.Relu, bias=ob1t[:])
                pp = psp.tile([D_OUT, NQ], F32, tag="ps")
                nc.tensor.matmul(pp[:], ow2[:], hT[:], start=True, stop=True)
                nc.scalar.activation(pTt[:, sl], pp[:], AF.Identity, bias=ob2t[:])
            nc.sync.dma_start(pT_d[:, :], pTt[:])

    nc.compile()
    return nc


_CACHE = {}
TRACE = False
_IOTA = np.arange(N_OBJ, dtype=np.float32)


def _get_nc():
    if "nc" not in _CACHE:
        _CACHE["nc"] = build()
    return _CACHE["nc"]


def _onehot_to_idx(a):
    """Exact index recovery from a one-hot float matrix via iota GEMV."""
    a = np.asarray(a)
    if a.dtype != np.float32:
        a = a.astype(np.float32)
    return a @ _IOTA


def _idx_blocks(v):
    """[N_REL] float indices -> [N_CORES*P, N_CHUNKS] f16, per-core chunk-major."""
    # per core: [E_PER_CORE] -> (N_CHUNKS, P) -> T -> [P, N_CHUNKS]
    return np.ascontiguousarray(
        np.transpose(v.reshape(N_CORES, N_CHUNKS, P), (0, 2, 1)).reshape(
            N_CORES * P, N_CHUNKS
        ),
        dtype=np.float16,
    )


def _get_runner():
    """Build (once) a cached jitted shard_map executable over the Bass NEFF."""
    if "runner" in _CACHE:
        return _CACHE["runner"]

    import jax
    from jax.experimental.shard_map import shard_map
    from jax.sharding import Mesh, NamedSharding, PartitionSpec

    from concourse.bass2jax import (
        _bass_exec_p,
        install_neuronx_cc_hook,
        partition_id_tensor,
    )

    nc = _get_nc()
    install_neuronx_cc_hook()
    partition_name = nc.partition_id_tensor.name if nc.partition_id_tensor else None
    in_names, out_names, out_avals, zero_outs = [], [], [], []
    for alloc in nc.m.functions[0].allocations:
        if not isinstance(alloc, mybir.MemoryLocationSet):
            continue
        name = alloc.memorylocations[0].name
        if alloc.kind == "ExternalInput":
            if name != partition_name:
                in_names.append(name)
        elif alloc.kind == "ExternalOutput":
            out_names.append(name)
            shape = tuple(alloc.tensor_shape)
            dtype = mybir.dt.np(alloc.dtype)
            out_avals.append(jax.core.ShapedArray(shape, dtype))
            zero_outs.append(np.zeros((N_CORES * shape[0], *shape[1:]), dtype))
    n_params = len(in_names)
    n_outs = len(out_avals)
    param_names = list(in_names)
    in_names = in_names + out_names
    if partition_name is not None:
        in_names.append(partition_name)
    # pT is fully written by the kernel, so the pre-zeroed output operand can
    # be uploaded once and reused (no donation)

    def _body(*args):
        operands = list(args)
        if partition_name is not None:
            operands.append(partition_id_tensor())
        outs = _bass_exec_p.bind(
            *operands,
            out_avals=tuple(out_avals),
            in_names=tuple(in_names),
            out_names=tuple(out_names),
            lowering_input_output_aliases=(),
            sim_require_finite=True,
            sim_require_nnan=True,
            nc=nc,
        )
        return tuple(outs)

    devices = jax.devices()[:N_CORES]
    mesh = Mesh(np.asarray(devices), ("core",))
    shard = NamedSharding(mesh, PartitionSpec("core"))
    repl = NamedSharding(mesh, PartitionSpec())
    param_specs = tuple(
        PartitionSpec("core") if n in _SHARDED_INPUTS else PartitionSpec()
        for n in param_names
    )
    in_specs = param_specs + (PartitionSpec("core"),) * n_outs
    out_specs = (PartitionSpec("core"),) * n_outs
    sharded = jax.jit(
        shard_map(_body, mesh=mesh, in_specs=in_specs, out_specs=out_specs,
                  check_rep=False),
        keep_unused=True,
    )
    zeros_dev = [jax.device_put(z, shard) for z in zero_outs]
    runner = dict(
        jax=jax, sharded=sharded, param_names=param_names,
        zeros_dev=zeros_dev, shard=shard, repl=repl, out_names=out_names,
    )
    _CACHE["runner"] = runner
    return runner


def kernel(**inputs):
    if not axon_active():
        return _kernel_fallback(**inputs)
    try:
        return _kernel_fast(**inputs)
    except Exception:
        _CACHE.pop("runner", None)
        return _kernel_fallback(**inputs)


_PROF = os.environ.get("KERNEL_PROF") == "1"


def _kernel_fast(**inputs):
    import time as _time
    _t = [_time.perf_counter()]
    r = _get_runner()
    jax = r["jax"]
    shard, repl = r["shard"], r["repl"]

    # 1) issue the index-independent uploads first (all async); the wire-heavy
    #    raT goes out before the host busies itself with the GEMVs
    devs = {}
    ra = np.asarray(inputs["ra"])
    raT = np.ascontiguousarray(
        np.transpose(ra.reshape(N_CORES, E_PER_CORE, D_REL), (0, 2, 1)).reshape(
            N_CORES * D_REL, E_PER_CORE
        ),
        dtype=np.float16,
    )
    devs["raT_c"] = jax.device_put(raT, shard)
    devs["obj"] = jax.device_put(
        np.asarray(inputs["obj"]).astype(np.float16).ravel(), shard)
    devs["wpack"] = jax.device_put(_pack_weights(inputs), shard)
    _t.append(time := __import__("time").perf_counter())

    # 2) index extraction overlaps with the transfers above
    recv = _onehot_to_idx(inputs["rr"])
    send = _onehot_to_idx(inputs["rs"])
    idx = np.concatenate([_idx_blocks(recv), _idx_blocks(send)], axis=1)
    devs["idx_c"] = jax.device_put(idx, shard)
    _t.append(__import__("time").perf_counter())

    # 3) dispatch + single sync, fetching only core 0's output shard
    out_arrs = r["sharded"](*[devs[n] for n in r["param_names"]], *r["zeros_dev"])
    _t.append(__import__("time").perf_counter())
    _CACHE["last_results"] = None
    pT0 = np.asarray(
        out_arrs[r["out_names"].index("pT")].addressable_shards[0].data
    )
    _t.append(__import__("time").perf_counter())
    if _PROF:
        d = [(_t[i + 1] - _t[i]) * 1e3 for i in range(len(_t) - 1)]
        print(f"[prof] puts {d[0]:.1f}  gemv+idx {d[1]:.1f}  disp {d[2]:.1f}  "
              f"sync {d[3]:.1f}  total {sum(d):.1f} ms")
    return np.ascontiguousarray(pT0.T)


def _kernel_fallback(**inputs):
    """Non-axon path: run through bass_utils with per-core input maps."""
    nc = _get_nc()
    objflat = np.asarray(inputs["obj"]).astype(np.float16).ravel()
    wpack = _pack_weights(inputs)
    recv = _onehot_to_idx(inputs["rr"])
    send = _onehot_to_idx(inputs["rs"])
    idx = np.concatenate([_idx_blocks(recv), _idx_blocks(send)], axis=1)
    ra = np.asarray(inputs["ra"])
    osh = objflat.size // N_CORES
    wsh = wpack.size // N_CORES
    in_maps = []
    for c in range(N_CORES):
        sl = slice(c * E_PER_CORE, (c + 1) * E_PER_CORE)
        m = {
            "obj": objflat[c * osh : (c + 1) * osh],
            "wpack": wpack[c * wsh : (c + 1) * wsh],
            "idx_c": np.ascontiguousarray(idx[c * P : (c + 1) * P, :]),
            "raT_c": np.ascontiguousarray(ra[sl].T, dtype=np.float16),
        }
        in_maps.append(m)
    res = run_bass_kernel_spmd(
        nc, in_maps, core_ids=list(range(N_CORES)), trace=TRACE
    )
    _CACHE["last_results"] = res
    return np.ascontiguousarray(res.results[0]["pT"].T)
